# revision 1
# baseline (speedup 1.0000x reference)
"""Trainium2 Bass kernel for nn_MultiHeadAttention (B=2, S=2048, E=1024, H=16).

Sharding: 8 NeuronCores = data-parallel over the 2 batches x tensor-parallel
over the 16 heads in 4 groups of 4 heads (Wq/Wk/Wv split column-wise, Wo
row-wise).  Each core computes a full-[S, E] partial of its batch's output;
the host sums the 4 head-group partials per batch.

Per-core device algorithm (S.T orientation so exp(S.T) feeds P@V directly):
  Q.T/K.T[n, s] = (wT chunk).T @ xT chunk      (e-outer, chases input DMAs)
  V[s, n]       = (xvT chunk).T @ wvT chunk    stored as v_ext = [V_h | ones]
  S.T_h[k, q]   = (K_h.T chunk).T @ Q_h.T      row-packed head pairs (d=64)
  P.T           = exp(S.T / 8)                 one ACT op per (k, head pair)
  [O.T_h; sums] = (v_ext_h).T @ P.T_h          fused: PSUM rows 0-63 = O.T_h,
                                               rows 64-127 = rowsum broadcast
  O.Tn_h        = O.T_h * recip(sums)          recip shifted p64->p0 via DMA
  out[m, :]     = sum_h (oT_h chunk).T @ woT_h

dtypes: matmul inputs for the projections are fp16 (host pre-cast halves the
HBM traffic; 10-bit mantissa beats bf16 by 8x); everything SBUF-internal
(Q.T/K.T/V/P.T/O.T/Wo) is float32r (full fp32 bits, reduced-precision
multiply); accumulation is always fp32.
"""

import numpy as np
from contextlib import ExitStack

import ml_dtypes

import concourse.bass as bass
import concourse.mybir as mybir
import concourse.tile as tile
from concourse.tile import ScopedClock
from concourse.bass_utils import run_bass_kernel_spmd

# ---------------------------------------------------------------------------
# Workarounds for the walrus build on this stack, which rejects more than ONE
# semaphore wait per instruction ("Too many sync wait commands").
# ---------------------------------------------------------------------------
_orig_commit_instruction = tile.TileContext._commit_instruction


def _commit_instruction(self, inst, lazy_reg_writes=True):
    si = getattr(inst, "sync_info", None)
    if si is not None and si.on_wait and len(si.on_wait) > 1:
        waits = list(si.on_wait)
        for w in waits[:-1]:
            nop = mybir.InstNoOp(
                name=self.nc.get_next_instruction_name(),
                ins=[], outs=[], engine=inst.engine,
            )
            nop.bass_nofuse = True
            nop.sync_info = mybir.SyncInfo(on_wait=[w], on_update=[])
            _orig_commit_instruction(self, nop, lazy_reg_writes=False)
        inst.sync_info = mybir.SyncInfo(
            on_wait=[waits[-1]], on_update=list(si.on_update or [])
        )
    return _orig_commit_instruction(self, inst, lazy_reg_writes)


def _drain_and_barrier(self, tick_clock, wait_clock):
    nc = self.nc
    drain_inst = nc.sync.drain()
    wait_clock.add_sem_waits(
        drain_inst.ins, ScopedClock({None: tick_clock.global_clock})
    )
    si = drain_inst.ins.sync_info
    waits = list(si.on_wait) if si and si.on_wait else []
    if len(waits) > 1:
        drain_inst.ins.sync_info = mybir.SyncInfo(
            on_wait=waits[:1], on_update=list(si.on_update or [])
        )
        for w in waits[1:]:
            extra = nc.sync.drain()
            esi = extra.ins.sync_info
            extra.ins.sync_info = mybir.SyncInfo(
                on_wait=[w],
                on_update=list(esi.on_update or []) if esi else [],
            )
    nc.all_engine_barrier()
    assert self.sems is not None
    popped = nc._tile_sem_poison_stack.pop()
    assert popped is self._sem_poison
    nc.clear_and_free_semaphores(list(self.sems.allocated().values()))
    nc.all_engine_barrier()


def _apply_tilefix():
    tile.TileContext._commit_instruction = _commit_instruction
    tile.TileContext._drain_and_barrier = _drain_and_barrier


_apply_tilefix()

# ---------------------------------------------------------------------------
# Problem constants (hardcoded)
# ---------------------------------------------------------------------------
B, S, E, H = 2, 2048, 1024, 16
HC, D = 4, 64              # heads per core, head dim
NCORES = 8
NE = E // 128              # 8  e-chunks
NQ = S // 512              # 4  q-chunks
NK = S // 128              # 16 k-chunks
NM = S // 128              # 16 m-chunks

F32 = mybir.dt.float32
BF16 = mybir.dt.bfloat16
FP16 = mybir.dt.float16


def build(mmdt=mybir.dt.float32r, pdt=mybir.dt.float32r, xdt=FP16,
          ovbufs=3, xbufs=1, shift_eng="scalar", ptbufs=6):
    nc = bass.Bass()
    xqT = nc.dram_tensor("xqT", [E, S], xdt, kind="ExternalInput")
    xkT = nc.dram_tensor("xkT", [E, S], xdt, kind="ExternalInput")
    xvT = nc.dram_tensor("xvT", [E, S], xdt, kind="ExternalInput")
    wqT = nc.dram_tensor("wqT", [E, 256], xdt, kind="ExternalInput")
    wkT = nc.dram_tensor("wkT", [E, 256], xdt, kind="ExternalInput")
    wvT = nc.dram_tensor("wvT", [E, 256], xdt, kind="ExternalInput")
    woT = nc.dram_tensor("woT", [256, E], mmdt, kind="ExternalInput")
    vones = nc.dram_tensor("vones", [128, 256], mmdt, kind="ExternalInput")
    out = nc.dram_tensor("out", [S, E], F32, kind="ExternalOutput")

    with tile.TileContext(nc) as tc, ExitStack() as ctx:
        consts = ctx.enter_context(tc.tile_pool(name="consts", bufs=1))
        wpool = ctx.enter_context(tc.tile_pool(name="w", bufs=1))
        actpool = ctx.enter_context(tc.tile_pool(name="acts", bufs=1))
        xpool = ctx.enter_context(tc.tile_pool(name="x", bufs=10))

        # preload the exp table before the hot loop
        dummy = consts.tile([1, 8], F32)
        nc.vector.memset(dummy[:], 0.0)
        nc.scalar.activation(dummy[:], dummy[:], mybir.ActivationFunctionType.Exp)

        wv_sb = wpool.tile([128, NE, 256], xdt)
        wo_sb = wpool.tile([64, HC, E], mmdt)

        qT_sb = actpool.tile([128, 2, S], mmdt)        # [(2 heads x d), pair, s]
        kT_sb = actpool.tile([128, 2, S], mmdt)
        v_sb = actpool.tile([128, NK, HC, 128], mmdt)  # [s%128, k, h, V_h|ones]

        def proj_eouter(w_sb, xchunks, dst, psA):
            tiles = [psA.tile([128, 512], F32, tag="mm", name=f"pj{i}")
                     for i in range(8)]
            for e in range(NE):
                for nch in range(2):
                    for m in range(NQ):
                        nc.tensor.matmul(
                            tiles[nch * NQ + m][:],
                            w_sb[:, e, nch * 128:(nch + 1) * 128],
                            xchunks[e][:, m * 512:(m + 1) * 512],
                            start=(e == 0), stop=(e == NE - 1),
                        )
            for nch in range(2):
                for m in range(NQ):
                    nc.vector.tensor_copy(
                        dst[:, nch, m * 512:(m + 1) * 512],
                        tiles[nch * NQ + m][:])

        # ---- prefix: K then Q projections (e-outer, DMA-chasing) ----
        with tc.tile_pool(name="wprefix", bufs=1) as wprefix, \
             tc.tile_pool(name="psA", bufs=8, space="PSUM") as psA:
            wk_sb = wprefix.tile([128, NE, 256], xdt)
            wq_sb = wprefix.tile([128, NE, 256], xdt)
            nc.sync.dma_start(wk_sb[:], wkT.rearrange("(ec p) n -> p ec n", p=128))
            nc.sync.dma_start(wq_sb[:], wqT.rearrange("(ec p) n -> p ec n", p=128))

            xk = []
            for e in range(NE):
                t = xpool.tile([128, S], xdt, tag="xchunk", name=f"xk{e}")
                nc.sync.dma_start(t[:], xkT[e * 128:(e + 1) * 128, :])
                xk.append(t)
            xq = []
            for e in range(NE):
                t = xpool.tile([128, S], xdt, tag="xchunk", name=f"xq{e}")
                nc.sync.dma_start(t[:], xqT[e * 128:(e + 1) * 128, :])
                xq.append(t)

            proj_eouter(wk_sb, xk, kT_sb, psA)
            proj_eouter(wq_sb, xq, qT_sb, psA)

        # V-side loads stream in behind the prefix on the SP queue
        nc.sync.dma_start(wv_sb[:], wvT.rearrange("(ec p) n -> p ec n", p=128))
        nc.sync.dma_start(wo_sb[:], woT.rearrange("(h p) j -> p h j", p=64))
        for k in range(NK):
            nc.gpsimd.dma_start(
                v_sb[:, k, :, 64:128],
                vones.rearrange("p (h c) -> p h c", h=HC))
        xv = []
        for e in range(NE):
            t = xpool.tile([128, S], xdt, tag="xchunk", name=f"xv{e}")
            nc.sync.dma_start(t[:], xvT[e * 128:(e + 1) * 128, :])
            xv.append(t)

        # ---- steady state pools ----
        oTpool = ctx.enter_context(tc.tile_pool(name="oT", bufs=1))
        ppool = ctx.enter_context(tc.tile_pool(name="pT", bufs=ptbufs))
        rpool = ctx.enter_context(tc.tile_pool(name="recip", bufs=2))
        opool = ctx.enter_context(tc.tile_pool(name="outstage", bufs=2))
        psS = ctx.enter_context(tc.tile_pool(name="psS", bufs=2, space="PSUM"))
        psOV = ctx.enter_context(tc.tile_pool(name="psOV", bufs=ovbufs, space="PSUM"))
        psX = ctx.enter_context(tc.tile_pool(name="psX", bufs=xbufs, space="PSUM"))

        oT_sb = oTpool.tile([64, HC, S], mmdt)         # [d, h, s]

        def v_proj_tile(m):
            ps = psX.tile([128, 512], F32, tag="px", name=f"vp{m}")
            for e in range(NE):
                nc.tensor.matmul(
                    ps[:, 0:256],
                    xv[e][:, m * 128:(m + 1) * 128],
                    wv_sb[:, e, :],
                    start=(e == 0), stop=(e == NE - 1),
                )
            nc.vector.tensor_copy(
                v_sb[:, m, :, 0:64],
                ps[:, 0:256].rearrange("p (h c) -> p h c", h=HC))

        def out_proj_tile(m):
            stage = opool.tile([128, E], F32)
            for j in range(2):
                ps = psX.tile([128, 512], F32, tag="px", name=f"op{m}_{j}")
                for h in range(HC):
                    nc.tensor.matmul(
                        ps[:],
                        oT_sb[:, h, m * 128:(m + 1) * 128],
                        wo_sb[:, h, j * 512:(j + 1) * 512],
                        start=(h == 0), stop=(h == HC - 1),
                    )
                nc.vector.tensor_copy(stage[:, j * 512:(j + 1) * 512], ps[:])
            nc.gpsimd.dma_start(out[m * 128:(m + 1) * 128, :], stage[:])

        # V tiles are needed from the very first pass: emit them first
        for m in range(NM):
            v_proj_tile(m)

        for qc in range(NQ):
            qs = slice(qc * 512, (qc + 1) * 512)
            for pair in range(2):
                ps_ov = [psOV.tile([128, 512], F32, name=f"ov{i}", tag="ov")
                         for i in range(2)]
                for k in range(NK):
                    ks = slice(k * 128, (k + 1) * 128)
                    first, last = (k == 0), (k == NK - 1)
                    ps_s = psS.tile([128, 1024], F32)
                    # scores, row-packed: head A rows 0-63, head B rows 64-127
                    nc.tensor.matmul(ps_s[:, 0:512],
                                     kT_sb[0:64, pair, ks],
                                     qT_sb[0:64, pair, qs],
                                     start=True, stop=True)
                    nc.tensor.matmul(ps_s[:, 512:1024],
                                     kT_sb[64:128, pair, ks],
                                     qT_sb[64:128, pair, qs],
                                     start=True, stop=True)
                    # exp of both heads in one op; 1/sqrt(D) folded into scale
                    pT = ppool.tile([128, 1024], pdt)
                    nc.scalar.activation(pT[:], ps_s[:],
                                         mybir.ActivationFunctionType.Exp,
                                         scale=0.125)
                    # fused O.T + rowsum accumulation per head
                    for h2 in range(2):
                        h = pair * 2 + h2
                        nc.tensor.matmul(
                            ps_ov[h2][:],
                            v_sb[:, k, h, :],
                            pT[:, h2 * 512:(h2 + 1) * 512],
                            start=first, stop=last)
                # normalize: recip of sums (rows 64-127), shift to rows 0-63
                for h2 in range(2):
                    h = pair * 2 + h2
                    rt = rpool.tile([128, 512], F32, tag="rt")
                    nc.vector.reciprocal(rt[64:128, :], ps_ov[h2][64:128, :])
                    rb = rpool.tile([64, 512], F32, tag="rb")
                    getattr(nc, shift_eng).dma_start(rb[:], rt[64:128, :])
                    nc.vector.tensor_tensor(
                        oT_sb[:, h, qs], ps_ov[h2][0:64, :], rb[:],
                        mybir.AluOpType.mult)
            # out-proj for this q window (needs both pairs of this qc)
            for m in range(qc * 4, qc * 4 + 4):
                out_proj_tile(m)

    return nc


_NC_CACHE = {}


def _get_nc():
    if "nc" not in _NC_CACHE:
        _NC_CACHE["nc"] = build()
    return _NC_CACHE["nc"]


def _shard_inputs(query, key, value, Wq, Wk, Wv, Wo):
    """Host-side sharding + layout prep: core c = (batch c//4, head-group c%4)."""
    f16 = np.float16
    xT = []
    for b in range(B):
        xT.append((
            np.ascontiguousarray(query[b].T).astype(f16),
            np.ascontiguousarray(key[b].T).astype(f16),
            np.ascontiguousarray(value[b].T).astype(f16),
        ))
    wT = []
    for g in range(4):
        gc = slice(g * 256, (g + 1) * 256)
        wT.append((
            np.ascontiguousarray(Wq[gc].T).astype(f16),
            np.ascontiguousarray(Wk[gc].T).astype(f16),
            np.ascontiguousarray(Wv[gc].T).astype(f16),
            np.ascontiguousarray(Wo[:, gc].T),
        ))
    vones = np.ones((128, 256), dtype=np.float32)
    in_maps = []
    for c in range(NCORES):
        b, g = c // 4, c % 4
        qT, kT, vT = xT[b]
        wq, wk, wv, wo = wT[g]
        in_maps.append({
            "xqT": qT, "xkT": kT, "xvT": vT,
            "wqT": wq, "wkT": wk, "wvT": wv, "woT": wo,
            "vones": vones,
        })
    return in_maps


def kernel(query, key, value, Wq, Wk, Wv, Wo):
    query = np.asarray(query, dtype=np.float32)
    key = np.asarray(key, dtype=np.float32)
    value = np.asarray(value, dtype=np.float32)
    Wq = np.asarray(Wq, dtype=np.float32)
    Wk = np.asarray(Wk, dtype=np.float32)
    Wv = np.asarray(Wv, dtype=np.float32)
    Wo = np.asarray(Wo, dtype=np.float32)

    nc = _get_nc()
    in_maps = _shard_inputs(query, key, value, Wq, Wk, Wv, Wo)
    res = run_bass_kernel_spmd(nc, in_maps, core_ids=list(range(NCORES)))

    out = np.zeros((B, S, E), dtype=np.float32)
    for c in range(NCORES):
        out[c // 4] += res.results[c]["out"]
    return out



# revision 9
# speedup vs baseline: 1.3386x; 1.3386x over previous
"""Trainium2 Bass kernel for nn_MultiHeadAttention (B=2, S=2048, E=1024, H=16).

Sharding: 8 NeuronCores = data-parallel over the 2 batches x tensor-parallel
over the 16 heads in 4 groups of 4 heads (Wq/Wk/Wv split column-wise, Wo
row-wise).  Each core computes a full-[S, E] partial of its batch's output;
the host sums the 4 head-group partials per batch.

Per-core device algorithm (S.T orientation feeds a flipped P@V):
  Q.T/K.T[n, s] = (wT chunk).T @ xT chunk       e/m-outer projections
  S.T_h[k, q]   = (K_h.T chunk).T @ Q_h.T       row-packed head pairs (d=64)
  P.T           = exp(S.T / 8)                  fp16, one ACT op per (k, pair)
  O[q, d|sum]   = (P.T chunk).T @ [V_h | 1]     FLIPPED: P.T [128k,128q] is the
                                                stationary, [V|ones] [128k,65]
                                                moving -> 65-col outputs, with
                                                the softmax denominator landing
                                                in column 64 (per-partition!)
  O_n           = O * recip(col 64)             one DVE tensor_scalar per tile
  O.T           = PE-transpose(O_n)             53ns/tile, restores [d, q]
  out[m, :]     = sum_pair (oT2 chunk).T @ woT2 contract-128 output projection

The flip + contract-128 out-proj cut PE matmul time ~27%; exp on the
Activation engine (~133us) becomes the critical resource, so the instruction
stream is interleaved to keep it saturated: each (qc, pair) "window" weaves
the NEXT window's scores+exp with THIS window's PV, plus V/Q-projection,
out-projection, and transpose filler work sized to the Act-engine pace.

dtypes: HBM traffic fp16 (in+out); scores fp32r; P/V/O/Wo fp16; accum fp32.
"""

import numpy as np
from contextlib import ExitStack

import ml_dtypes

import concourse.bass as bass
import concourse.mybir as mybir
import concourse.tile as tile
from concourse.tile import ScopedClock
from concourse.bass_utils import run_bass_kernel_spmd

# ---------------------------------------------------------------------------
# Workarounds for the walrus build on this stack, which rejects more than ONE
# semaphore wait per instruction ("Too many sync wait commands").
# ---------------------------------------------------------------------------
_orig_commit_instruction = tile.TileContext._commit_instruction


def _commit_instruction(self, inst, lazy_reg_writes=True):
    si = getattr(inst, "sync_info", None)
    if si is not None and si.on_wait and len(si.on_wait) > 1:
        waits = list(si.on_wait)
        for w in waits[:-1]:
            nop = mybir.InstNoOp(
                name=self.nc.get_next_instruction_name(),
                ins=[], outs=[], engine=inst.engine,
            )
            nop.bass_nofuse = True
            nop.sync_info = mybir.SyncInfo(on_wait=[w], on_update=[])
            _orig_commit_instruction(self, nop, lazy_reg_writes=False)
        inst.sync_info = mybir.SyncInfo(
            on_wait=[waits[-1]], on_update=list(si.on_update or [])
        )
    return _orig_commit_instruction(self, inst, lazy_reg_writes)


def _drain_and_barrier(self, tick_clock, wait_clock):
    nc = self.nc
    drain_inst = nc.sync.drain()
    wait_clock.add_sem_waits(
        drain_inst.ins, ScopedClock({None: tick_clock.global_clock})
    )
    si = drain_inst.ins.sync_info
    waits = list(si.on_wait) if si and si.on_wait else []
    if len(waits) > 1:
        drain_inst.ins.sync_info = mybir.SyncInfo(
            on_wait=waits[:1], on_update=list(si.on_update or [])
        )
        for w in waits[1:]:
            extra = nc.sync.drain()
            esi = extra.ins.sync_info
            extra.ins.sync_info = mybir.SyncInfo(
                on_wait=[w],
                on_update=list(esi.on_update or []) if esi else [],
            )
    nc.all_engine_barrier()
    assert self.sems is not None
    popped = nc._tile_sem_poison_stack.pop()
    assert popped is self._sem_poison
    nc.clear_and_free_semaphores(list(self.sems.allocated().values()))
    nc.all_engine_barrier()


def _apply_tilefix():
    tile.TileContext._commit_instruction = _commit_instruction
    tile.TileContext._drain_and_barrier = _drain_and_barrier


_apply_tilefix()

# ---------------------------------------------------------------------------
# Problem constants (hardcoded)
# ---------------------------------------------------------------------------
B, S, E, H = 2, 2048, 1024, 16
HC, D = 4, 64              # heads per core, head dim
NCORES = 8
NE = E // 128              # 8  e-chunks (projection contraction)
NK = S // 128              # 16 k-chunks
NQC = S // 512             # 4  512-wide q windows
NM = S // 128              # 16 output row chunks

F32 = mybir.dt.float32
F32R = mybir.dt.float32r
FP16 = mybir.dt.float16
EXP = mybir.ActivationFunctionType.Exp
MULT = mybir.AluOpType.mult


def build(ptbufs=18, xdt=FP16, qkdt=mybir.dt.float32r):
    nc = bass.Bass()
    xqT = nc.dram_tensor("xqT", [E, S], xdt, kind="ExternalInput")
    xkT = nc.dram_tensor("xkT", [E, S], xdt, kind="ExternalInput")
    xvT = nc.dram_tensor("xvT", [E, S], xdt, kind="ExternalInput")
    wqT = nc.dram_tensor("wqT", [E, 256], xdt, kind="ExternalInput")
    wkT = nc.dram_tensor("wkT", [E, 256], xdt, kind="ExternalInput")
    wvT = nc.dram_tensor("wvT", [E, 256], xdt, kind="ExternalInput")
    woT2 = nc.dram_tensor("woT2", [128, 2 * E], FP16, kind="ExternalInput")
    ident = nc.dram_tensor("ident", [128, 128], FP16, kind="ExternalInput")
    out = nc.dram_tensor("out", [S, E], FP16, kind="ExternalOutput")

    with tile.TileContext(nc) as tc, ExitStack() as ctx:
        consts = ctx.enter_context(tc.tile_pool(name="consts", bufs=1))
        wpool = ctx.enter_context(tc.tile_pool(name="w", bufs=1))
        actpool = ctx.enter_context(tc.tile_pool(name="acts", bufs=1))
        xkvpool = ctx.enter_context(tc.tile_pool(name="xkv", bufs=10))
        xqpool = ctx.enter_context(tc.tile_pool(name="xq", bufs=32))

        # preload the exp table before the hot loop
        dummy = consts.tile([1, 8], F32)
        nc.vector.memset(dummy[:], 0.0)
        nc.scalar.activation(dummy[:], dummy[:], EXP)

        wq_sb = wpool.tile([128, NE, 256], xdt)
        wk_sb = wpool.tile([128, NE, 256], xdt)
        wv_sb = wpool.tile([128, NE, 256], xdt)
        wo2_sb = wpool.tile([128, 2, E], FP16)
        id_sb = wpool.tile([128, 128], FP16)

        qT_sb = actpool.tile([128, 2, S], qkdt)        # [(2 heads x d), pair, s]
        kT_sb = actpool.tile([128, 2, S], qkdt)
        v_sb = actpool.tile([128, NK, HC, 65], FP16)   # [s%128, k, h, V_h|one]
        oT2_sb = actpool.tile([128, 2, S], FP16)       # [(2 heads x d), pair, q]

        # ---- DMA program: ordered by first-use ----
        nc.sync.dma_start(wk_sb[:], wkT.rearrange("(ec p) n -> p ec n", p=128))
        xk = []
        for e in range(NE):
            t = xkvpool.tile([128, S], xdt, tag="xc", name=f"xk{e}")
            nc.sync.dma_start(t[:], xkT[e * 128:(e + 1) * 128, :])
            xk.append(t)
        nc.sync.dma_start(wq_sb[:], wqT.rearrange("(ec p) n -> p ec n", p=128))
        # xq arrives in per-qc column slices so qc0 lands early
        xq = [[None] * NE for _ in range(NQC)]
        for qc in [0]:
            for e in range(NE):
                t = xqpool.tile([128, 512], xdt, tag="xq", name=f"xq{qc}_{e}")
                nc.sync.dma_start(
                    t[:], xqT[e * 128:(e + 1) * 128, qc * 512:(qc + 1) * 512])
                xq[qc][e] = t
        nc.sync.dma_start(wv_sb[:], wvT.rearrange("(ec p) n -> p ec n", p=128))
        xv = []
        for e in range(NE):
            t = xkvpool.tile([128, S], xdt, tag="xc", name=f"xv{e}")
            nc.sync.dma_start(t[:], xvT[e * 128:(e + 1) * 128, :])
            xv.append(t)
        nc.sync.dma_start(
            wo2_sb[:], woT2.rearrange("p (two e) -> p two e", two=2))
        nc.sync.dma_start(id_sb[:], ident[0:128, :])
        for qc in range(1, NQC):
            for e in range(NE):
                t = xqpool.tile([128, 512], xdt, tag="xq", name=f"xq{qc}_{e}")
                nc.sync.dma_start(
                    t[:], xqT[e * 128:(e + 1) * 128, qc * 512:(qc + 1) * 512])
                xq[qc][e] = t

        # ones column for the fused rowsum (PV moving operand col 64)
        nc.vector.memset(v_sb[:, :, :, 64:65], 1.0)

        # ---- prefix: K projection (e-outer, chases the xk DMAs) + Q qc0 ----
        with tc.tile_pool(name="psA", bufs=8, space="PSUM") as psA:
            ktiles = [psA.tile([128, 512], F32, tag="mm", name=f"pk{i}")
                      for i in range(8)]
            for e in range(NE):
                for nch in range(2):
                    for m in range(4):
                        nc.tensor.matmul(
                            ktiles[nch * 4 + m][:],
                            wk_sb[:, e, nch * 128:(nch + 1) * 128],
                            xk[e][:, m * 512:(m + 1) * 512],
                            start=(e == 0), stop=(e == NE - 1),
                        )
            for nch in range(2):
                for m in range(4):
                    nc.vector.tensor_copy(
                        kT_sb[:, nch, m * 512:(m + 1) * 512],
                        ktiles[nch * 4 + m][:])
            for nch in range(2):
                t = psA.tile([128, 512], F32, tag="mm", name=f"pq0_{nch}")
                for e in range(NE):
                    nc.tensor.matmul(
                        t[:],
                        wq_sb[:, e, nch * 128:(nch + 1) * 128],
                        xq[0][e][:],
                        start=(e == 0), stop=(e == NE - 1),
                    )
                nc.vector.tensor_copy(qT_sb[:, nch, 0:512], t[:])

        # ---- steady-state pools ----
        psS = ctx.enter_context(tc.tile_pool(name="psS", bufs=2, space="PSUM"))
        psOV = ctx.enter_context(tc.tile_pool(name="psOV", bufs=1, space="PSUM"))
        psM = ctx.enter_context(tc.tile_pool(name="psM", bufs=2, space="PSUM"))
        ppool = ctx.enter_context(tc.tile_pool(name="pT", bufs=ptbufs))
        npool = ctx.enter_context(tc.tile_pool(name="nT", bufs=6))
        rpool = ctx.enter_context(tc.tile_pool(name="rt", bufs=4))
        stpool = ctx.enter_context(tc.tile_pool(name="stage", bufs=2))

        def emit_scores_exp(qc, pair, k):
            qs = slice(qc * 512, (qc + 1) * 512)
            ks = slice(k * 128, (k + 1) * 128)
            ps_s = psS.tile([128, 1024], F32, tag="ss", name=f"ss{qc}_{pair}_{k}")
            nc.tensor.matmul(ps_s[:, 0:512],
                             kT_sb[0:64, pair, ks], qT_sb[0:64, pair, qs],
                             start=True, stop=True)
            nc.tensor.matmul(ps_s[:, 512:1024],
                             kT_sb[64:128, pair, ks], qT_sb[64:128, pair, qs],
                             start=True, stop=True)
            pT = ppool.tile([128, 1024], FP16, tag="pt", name=f"pt{qc}_{pair}_{k}")
            nc.scalar.activation(pT[:], ps_s[:], EXP, scale=0.125)
            return pT

        def emit_pv(pair, k, pT, ovv):
            # ovv: ps_o viewed as [128, h2, qt, 128]; O in cols 0:64, sum col 64
            # PSUM zero regions are 2KB: start=True pending-zeroes the WHOLE
            # bank, so only the first region per bank may start the group; the
            # other regions' k==0 writes land on pending-zero bytes and
            # write-through (hardware lazy-zero semantics).
            for h2 in range(2):
                h = pair * 2 + h2
                for qt in range(4):
                    nc.tensor.matmul(
                        ovv[:, h2, qt, 0:65],
                        pT[:, h2 * 512 + qt * 128:h2 * 512 + (qt + 1) * 128],
                        v_sb[:, k, h, 0:65],
                        start=(k == 0 and qt == 0), stop=(k == NK - 1),
                        skip_group_check=True,
                    )

        def emit_vproj(k):
            t = psM.tile([128, 512], F32, tag="mi", name=f"vp{k}")
            for e in range(NE):
                nc.tensor.matmul(
                    t[:, 0:256],
                    xv[e][:, k * 128:(k + 1) * 128],
                    wv_sb[:, e, :],
                    start=(e == 0), stop=(e == NE - 1),
                )
            nc.vector.tensor_copy(
                v_sb[:, k, :, 0:64],
                t[:, 0:256].rearrange("p (h c) -> p h c", h=HC))

        def emit_qproj(qc, nch):
            t = psM.tile([128, 512], F32, tag="mi", name=f"qp{qc}_{nch}")
            for e in range(NE):
                nc.tensor.matmul(
                    t[:],
                    wq_sb[:, e, nch * 128:(nch + 1) * 128],
                    xq[qc][e][:],
                    start=(e == 0), stop=(e == NE - 1),
                )
            nc.vector.tensor_copy(qT_sb[:, nch, qc * 512:(qc + 1) * 512], t[:])

        def emit_norm(qc, pair, ovv):
            # reciprocal of the fused rowsums (col 64 of each 128-col region)
            rt = rpool.tile([128, 8], F32, tag="rt", name=f"rt{qc}_{pair}")
            for h2 in range(2):
                nc.vector.reciprocal(rt[:, h2 * 4:(h2 + 1) * 4],
                                     ovv[:, h2, :, 64])
            nTs = []
            for qt in range(4):
                nT = npool.tile([128, 128], FP16, tag="nt",
                                name=f"nt{qc}_{pair}_{qt}")
                for h2 in range(2):
                    nc.vector.tensor_scalar(
                        nT[:, h2 * 64:(h2 + 1) * 64],
                        ovv[:, h2, qt, 0:64],
                        rt[:, h2 * 4 + qt:h2 * 4 + qt + 1],
                        None, MULT)
                nTs.append(nT)
            return nTs

        def emit_transpose(qc, pair, qt, nT):
            psT = psM.tile([128, 128], FP16, tag="mi", name=f"tp{qc}_{pair}_{qt}")
            nc.tensor.transpose(psT[:], nT[:], id_sb[:])
            nc.vector.tensor_copy(
                oT2_sb[:, pair, qc * 512 + qt * 128:qc * 512 + (qt + 1) * 128],
                psT[:])

        def emit_outproj(m):
            stage = stpool.tile([128, E], FP16, tag="st", name=f"st{m}")
            for j in range(2):
                t = psM.tile([128, 512], F32, tag="mi", name=f"op{m}_{j}")
                for pair in range(2):
                    nc.tensor.matmul(
                        t[:],
                        oT2_sb[:, pair, m * 128:(m + 1) * 128],
                        wo2_sb[:, pair, j * 512:(j + 1) * 512],
                        start=(pair == 0), stop=(pair == 1),
                    )
                nc.vector.tensor_copy(stage[:, j * 512:(j + 1) * 512], t[:])
            nc.gpsimd.dma_start(out[m * 128:(m + 1) * 128, :], stage[:])

        # ---- window-pipelined attention ----
        # Stage 0: pure A-phase for (0,0): scores+exp, V-proj woven late so the
        # PE never head-blocks on the xv DMAs.
        pts = {}
        cur = [emit_scores_exp(0, 0, k) for k in range(9)]
        vdone = 0
        for k in range(9, NK):
            cur.append(emit_scores_exp(0, 0, k))
            emit_vproj(vdone)
            vdone += 1
        pts[(0, 0)] = cur

        seq = [(qc, pair) for qc in range(NQC) for pair in range(2)]
        trans_pending = None   # (qc, pair, nTs) awaiting transpose weave
        for i, (qc, pair) in enumerate(seq):
            nxt = seq[i + 1] if i + 1 < len(seq) else None
            ps_o = psOV.tile([128, 1024], F32, tag="ov", name=f"ov{qc}_{pair}")
            ovv = ps_o.rearrange("p (h q c) -> p h q c", h=2, q=4)
            cur_pts = pts.pop((qc, pair))
            nxt_pts = [] if nxt else None
            op_ms = list(range((qc - 1) * 4, qc * 4)) if (pair == 0 and qc >= 1) else []
            for k in range(NK):
                if nxt:
                    nxt_pts.append(emit_scores_exp(nxt[0], nxt[1], k))
                if trans_pending and k < 4:
                    tqc, tpair, tnTs = trans_pending
                    emit_transpose(tqc, tpair, k, tnTs[k])
                    if k == 3:
                        trans_pending = None
                if vdone < NK:  # remaining V-proj tiles, just-in-time
                    emit_vproj(vdone)
                    vdone += 1
                emit_pv(pair, k, cur_pts[k], ovv)
                if op_ms and k % 4 == 3:
                    emit_outproj(op_ms[k // 4])
                if pair == 0 and qc < NQC - 1 and k in (8, 12):
                    # Q projection for the next qc, before its scores appear
                    emit_qproj(qc + 1, (k - 8) // 4)
            if nxt:
                pts[nxt] = nxt_pts
            nTs = emit_norm(qc, pair, ovv)
            trans_pending = (qc, pair, nTs)

        # tail: last window's transposes + final out-projection chunk
        tqc, tpair, tnTs = trans_pending
        for qt in range(4):
            emit_transpose(tqc, tpair, qt, tnTs[qt])
        for m in range((NQC - 1) * 4, NM):
            emit_outproj(m)

    return nc


_NC_CACHE = {}


def _get_nc():
    if "nc" not in _NC_CACHE:
        _NC_CACHE["nc"] = build()
    return _NC_CACHE["nc"]


def _shard_inputs(query, key, value, Wq, Wk, Wv, Wo):
    """Host-side sharding + layout prep: core c = (batch c//4, head-group c%4)."""
    f16 = np.float16
    xT = []
    for b in range(B):
        xT.append((
            np.ascontiguousarray(query[b].T).astype(f16),
            np.ascontiguousarray(key[b].T).astype(f16),
            np.ascontiguousarray(value[b].T).astype(f16),
        ))
    wT = []
    for g in range(4):
        gc = slice(g * 256, (g + 1) * 256)
        wo_g = Wo[:, gc].T.astype(f16)            # [256, E]
        woT2 = np.ascontiguousarray(
            wo_g.reshape(2, 128, E).transpose(1, 0, 2).reshape(128, 2 * E))
        wT.append((
            np.ascontiguousarray(Wq[gc].T).astype(f16),
            np.ascontiguousarray(Wk[gc].T).astype(f16),
            np.ascontiguousarray(Wv[gc].T).astype(f16),
            woT2,
        ))
    ident = np.eye(128, dtype=f16)
    in_maps = []
    for c in range(NCORES):
        b, g = c // 4, c % 4
        qT, kT, vT = xT[b]
        wq, wk, wv, wo2 = wT[g]
        in_maps.append({
            "xqT": qT, "xkT": kT, "xvT": vT,
            "wqT": wq, "wkT": wk, "wvT": wv, "woT2": wo2,
            "ident": ident,
        })
    return in_maps


def kernel(query, key, value, Wq, Wk, Wv, Wo):
    query = np.asarray(query, dtype=np.float32)
    key = np.asarray(key, dtype=np.float32)
    value = np.asarray(value, dtype=np.float32)
    Wq = np.asarray(Wq, dtype=np.float32)
    Wk = np.asarray(Wk, dtype=np.float32)
    Wv = np.asarray(Wv, dtype=np.float32)
    Wo = np.asarray(Wo, dtype=np.float32)

    nc = _get_nc()
    in_maps = _shard_inputs(query, key, value, Wq, Wk, Wv, Wo)
    res = run_bass_kernel_spmd(nc, in_maps, core_ids=list(range(NCORES)))

    out = np.zeros((B, S, E), dtype=np.float32)
    for c in range(NCORES):
        out[c // 4] += res.results[c]["out"].astype(np.float32)
    return out


# revision 22
# speedup vs baseline: 1.3768x; 1.0285x over previous
"""Trainium2 Bass kernel for nn_MultiHeadAttention (B=2, S=2048, E=1024, H=16).

Sharding: 8 NeuronCores = data-parallel over the 2 batches x tensor-parallel
over the 16 heads in 4 groups of 4 heads (Wq/Wk/Wv split column-wise, Wo
row-wise).  Each core computes a full-[S, E] partial of its batch's output;
the host sums the 4 head-group partials per batch.

Per-core device algorithm (S.T orientation feeds a flipped P@V):
  Q.T/K.T[n, s] = (wT chunk).T @ xT chunk       e/m-outer projections
  S.T_h[k, q]   = (K_h.T chunk).T @ Q_h.T       row-packed head pairs (d=64)
  P.T           = exp(S.T / 8)                  fp16, one ACT op per (k, pair)
  O[q, d|sum]   = (P.T chunk).T @ [V_h | 1]     FLIPPED: P.T [128k,128q] is the
                                                stationary, [V|ones] [128k,65]
                                                moving -> 65-col outputs, with
                                                the softmax denominator landing
                                                in column 64 (per-partition!)
  O_n           = O * recip(col 64)             one DVE tensor_scalar per tile
  O.T           = PE-transpose(O_n)             53ns/tile, restores [d, q]
  out[m, :]     = sum_pair (oT2 chunk).T @ woT2 contract-128 output projection

The flip + contract-128 out-proj cut PE matmul time ~27%; exp on the
Activation engine (~133us) becomes the critical resource, so the instruction
stream is interleaved to keep it saturated: each (qc, pair) "window" weaves
the NEXT window's scores+exp with THIS window's PV, plus V/Q-projection,
out-projection, and transpose filler work sized to the Act-engine pace.

dtypes: HBM traffic fp16 (in+out); scores fp32r; P/V/O/Wo fp16; accum fp32.
"""

import numpy as np
from contextlib import ExitStack

import ml_dtypes

import concourse.bass as bass
import concourse.mybir as mybir
import concourse.tile as tile
from concourse.tile import ScopedClock
from concourse.bass_utils import run_bass_kernel_spmd

# ---------------------------------------------------------------------------
# Workarounds for the walrus build on this stack, which rejects more than ONE
# semaphore wait per instruction ("Too many sync wait commands").
# ---------------------------------------------------------------------------
_orig_commit_instruction = tile.TileContext._commit_instruction


def _commit_instruction(self, inst, lazy_reg_writes=True):
    si = getattr(inst, "sync_info", None)
    if si is not None and si.on_wait and len(si.on_wait) > 1:
        waits = list(si.on_wait)
        for w in waits[:-1]:
            nop = mybir.InstNoOp(
                name=self.nc.get_next_instruction_name(),
                ins=[], outs=[], engine=inst.engine,
            )
            nop.bass_nofuse = True
            nop.sync_info = mybir.SyncInfo(on_wait=[w], on_update=[])
            _orig_commit_instruction(self, nop, lazy_reg_writes=False)
        inst.sync_info = mybir.SyncInfo(
            on_wait=[waits[-1]], on_update=list(si.on_update or [])
        )
    return _orig_commit_instruction(self, inst, lazy_reg_writes)


def _drain_and_barrier(self, tick_clock, wait_clock):
    nc = self.nc
    drain_inst = nc.sync.drain()
    wait_clock.add_sem_waits(
        drain_inst.ins, ScopedClock({None: tick_clock.global_clock})
    )
    si = drain_inst.ins.sync_info
    waits = list(si.on_wait) if si and si.on_wait else []
    if len(waits) > 1:
        drain_inst.ins.sync_info = mybir.SyncInfo(
            on_wait=waits[:1], on_update=list(si.on_update or [])
        )
        for w in waits[1:]:
            extra = nc.sync.drain()
            esi = extra.ins.sync_info
            extra.ins.sync_info = mybir.SyncInfo(
                on_wait=[w],
                on_update=list(esi.on_update or []) if esi else [],
            )
    nc.all_engine_barrier()
    assert self.sems is not None
    popped = nc._tile_sem_poison_stack.pop()
    assert popped is self._sem_poison
    nc.clear_and_free_semaphores(list(self.sems.allocated().values()))
    nc.all_engine_barrier()


def _apply_tilefix():
    tile.TileContext._commit_instruction = _commit_instruction
    tile.TileContext._drain_and_barrier = _drain_and_barrier


_apply_tilefix()

# ---------------------------------------------------------------------------
# Problem constants (hardcoded)
# ---------------------------------------------------------------------------
B, S, E, H = 2, 2048, 1024, 16
HC, D = 4, 64              # heads per core, head dim
NCORES = 8
NE = E // 128              # 8  e-chunks (projection contraction)
NK = S // 128              # 16 k-chunks
NQC = S // 512             # 4  512-wide q windows
NM = S // 128              # 16 output row chunks

F32 = mybir.dt.float32
F32R = mybir.dt.float32r
FP16 = mybir.dt.float16
EXP = mybir.ActivationFunctionType.Exp
MULT = mybir.AluOpType.mult


def build(ptbufs=18, xdt=FP16, qkdt=mybir.dt.float32r):
    nc = bass.Bass()
    xqT = nc.dram_tensor("xqT", [E, S], xdt, kind="ExternalInput")
    xkT = nc.dram_tensor("xkT", [E, S], xdt, kind="ExternalInput")
    xvT = nc.dram_tensor("xvT", [E, S], xdt, kind="ExternalInput")
    wqT = nc.dram_tensor("wqT", [E, 256], xdt, kind="ExternalInput")
    wkT = nc.dram_tensor("wkT", [E, 256], xdt, kind="ExternalInput")
    wvT = nc.dram_tensor("wvT", [E, 256], xdt, kind="ExternalInput")
    woT2 = nc.dram_tensor("woT2", [128, 2 * E], FP16, kind="ExternalInput")
    ident = nc.dram_tensor("ident", [128, 128], FP16, kind="ExternalInput")
    out = nc.dram_tensor("out", [S, E], FP16, kind="ExternalOutput")

    with tile.TileContext(nc) as tc, ExitStack() as ctx:
        consts = ctx.enter_context(tc.tile_pool(name="consts", bufs=1))
        wpool = ctx.enter_context(tc.tile_pool(name="w", bufs=1))
        actpool = ctx.enter_context(tc.tile_pool(name="acts", bufs=1))
        xkvpool = ctx.enter_context(tc.tile_pool(name="xkv", bufs=10))
        xqpool = ctx.enter_context(tc.tile_pool(name="xq", bufs=32))

        # preload the exp table before the hot loop
        dummy = consts.tile([1, 8], F32)
        nc.vector.memset(dummy[:], 0.0)
        nc.scalar.activation(dummy[:], dummy[:], EXP)

        wq_sb = wpool.tile([128, NE, 256], xdt)
        wk_sb = wpool.tile([128, NE, 256], xdt)
        wv_sb = wpool.tile([128, NE, 256], xdt)
        wo2_sb = wpool.tile([128, 2, E], FP16)
        id_sb = wpool.tile([128, 128], FP16)

        # per-pair tensors: tile-granular dependency tracking means a read
        # waits on every writer emitted so far, so pair0's scores must not
        # share a tile with pair1's (later-copied) projections
        qT_sb = [actpool.tile([128, S], qkdt, name=f"qT{p}") for p in range(2)]
        kT_sb = [actpool.tile([128, S], qkdt, name=f"kT{p}") for p in range(2)]
        v_sb = actpool.tile([128, NK, HC, 65], FP16)   # [s%128, k, h, V_h|one]
        oT2_sb = actpool.tile([128, 2, S], FP16)       # [(2 heads x d), pair, q]

        # ---- DMA program: ordered by first-use ----
        nc.sync.dma_start(wk_sb[:], wkT.rearrange("(ec p) n -> p ec n", p=128))
        xk = []
        for e in range(NE):
            t = xkvpool.tile([128, S], xdt, tag="xc", name=f"xk{e}")
            nc.sync.dma_start(t[:], xkT[e * 128:(e + 1) * 128, :])
            xk.append(t)
        nc.sync.dma_start(wq_sb[:], wqT.rearrange("(ec p) n -> p ec n", p=128))
        # xq arrives in per-qc column slices so qc0 lands early
        xq = [[None] * NE for _ in range(NQC)]
        for qc in [0]:
            for e in range(NE):
                t = xqpool.tile([128, 512], xdt, tag="xq", name=f"xq{qc}_{e}")
                nc.sync.dma_start(
                    t[:], xqT[e * 128:(e + 1) * 128, qc * 512:(qc + 1) * 512])
                xq[qc][e] = t
        nc.sync.dma_start(wv_sb[:], wvT.rearrange("(ec p) n -> p ec n", p=128))
        xv = []
        for e in range(NE):
            t = xkvpool.tile([128, S], xdt, tag="xc", name=f"xv{e}")
            nc.sync.dma_start(t[:], xvT[e * 128:(e + 1) * 128, :])
            xv.append(t)
        nc.sync.dma_start(
            wo2_sb[:], woT2.rearrange("p (two e) -> p two e", two=2))
        nc.sync.dma_start(id_sb[:], ident[0:128, :])
        for qc in range(1, NQC):
            for e in range(NE):
                t = xqpool.tile([128, 512], xdt, tag="xq", name=f"xq{qc}_{e}")
                nc.sync.dma_start(
                    t[:], xqT[e * 128:(e + 1) * 128, qc * 512:(qc + 1) * 512])
                xq[qc][e] = t

        # ones column for the fused rowsum (PV moving operand col 64)
        nc.vector.memset(v_sb[:, :, :, 64:65], 1.0)

        # ---- steady-state pools (psS first: it coexists with prefix psA) ----
        psS = ctx.enter_context(tc.tile_pool(name="psS", bufs=2, space="PSUM"))
        ppool = ctx.enter_context(tc.tile_pool(name="pT", bufs=ptbufs))
        npool = ctx.enter_context(tc.tile_pool(name="nT", bufs=6))
        rpool = ctx.enter_context(tc.tile_pool(name="rt", bufs=4))
        stpool = ctx.enter_context(tc.tile_pool(name="stage", bufs=2))

        def emit_scores_exp(qc, pair, k):
            qs = slice(qc * 512, (qc + 1) * 512)
            ks = slice(k * 128, (k + 1) * 128)
            ps_s = psS.tile([128, 1024], F32, tag="ss", name=f"ss{qc}_{pair}_{k}")
            nc.tensor.matmul(ps_s[:, 0:512],
                             kT_sb[pair][0:64, ks], qT_sb[pair][0:64, qs],
                             start=True, stop=True)
            nc.tensor.matmul(ps_s[:, 512:1024],
                             kT_sb[pair][64:128, ks], qT_sb[pair][64:128, qs],
                             start=True, stop=True)
            pT = ppool.tile([128, 1024], FP16, tag="pt", name=f"pt{qc}_{pair}_{k}")
            nc.scalar.activation(pT[:], ps_s[:], EXP, scale=0.125)
            return pT

        def emit_pv(pair, k, pT, ovv):
            # ovv: ps_o viewed as [128, h2, qt, 128]; O in cols 0:64, sum col 64
            # PSUM zero regions are 2KB: start=True pending-zeroes the WHOLE
            # bank, so only the first region per bank may start the group; the
            # other regions' k==0 writes land on pending-zero bytes and
            # write-through (hardware lazy-zero semantics).
            for h2 in range(2):
                h = pair * 2 + h2
                for qt in range(4):
                    nc.tensor.matmul(
                        ovv[:, h2, qt, 0:65],
                        pT[:, h2 * 512 + qt * 128:h2 * 512 + (qt + 1) * 128],
                        v_sb[:, k, h, 0:65],
                        start=(k == 0 and qt == 0), stop=(k == NK - 1),
                        skip_group_check=True,
                    )

        def emit_vproj(k):
            t = psM.tile([128, 512], F32, tag="mi", name=f"vp{k}")
            for e in range(NE):
                nc.tensor.matmul(
                    t[:, 0:256],
                    xv[e][:, k * 128:(k + 1) * 128],
                    wv_sb[:, e, :],
                    start=(e == 0), stop=(e == NE - 1),
                )
            nc.vector.tensor_copy(
                v_sb[:, k, :, 0:64],
                t[:, 0:256].rearrange("p (h c) -> p h c", h=HC))

        def emit_qproj(qc, nch):
            t = psM.tile([128, 512], F32, tag="mi", name=f"qp{qc}_{nch}")
            for e in range(NE):
                nc.tensor.matmul(
                    t[:],
                    wq_sb[:, e, nch * 128:(nch + 1) * 128],
                    xq[qc][e][:],
                    start=(e == 0), stop=(e == NE - 1),
                )
            nc.vector.tensor_copy(qT_sb[nch][:, qc * 512:(qc + 1) * 512], t[:])

        def emit_norm(qc, pair, ovv, split=False):
            # reciprocal of the fused rowsums (col 64 of each 128-col region)
            rt = rpool.tile([128, 8], F32, tag="rt", name=f"rt{qc}_{pair}")
            for h2 in range(2):
                nc.vector.reciprocal(rt[:, h2 * 4:(h2 + 1) * 4],
                                     ovv[:, h2, :, 64])
            nTs = []
            for qt in range(4):
                nT = npool.tile([128, 128], FP16, tag="nt",
                                name=f"nt{qc}_{pair}_{qt}")
                for h2 in range(2):
                    if split and h2 == 1:
                        # tail only: Act engine is idle, halve the DVE chain
                        nc.scalar.mul(nT[:, 64:128], ovv[:, 1, qt, 0:64],
                                      rt[:, 4 + qt:5 + qt])
                    else:
                        nc.vector.tensor_scalar(
                            nT[:, h2 * 64:(h2 + 1) * 64],
                            ovv[:, h2, qt, 0:64],
                            rt[:, h2 * 4 + qt:h2 * 4 + qt + 1],
                            None, MULT)
                nTs.append(nT)
            return nTs

        def emit_transpose(qc, pair, qt, nT, pool=None):
            # pool=psS in the tail: scores are done, its slots are idle, and
            # keeping transposes out of the psM ring stops them serializing
            # behind out-proj tiles awaiting their stage copies.
            pool = pool or psM
            tag = "ss" if pool is psS else "mi"
            psT = pool.tile([128, 128], FP16, tag=tag, name=f"tp{qc}_{pair}_{qt}")
            nc.tensor.transpose(psT[:], nT[:], id_sb[:])
            nc.vector.tensor_copy(
                oT2_sb[:, pair, qc * 512 + qt * 128:qc * 512 + (qt + 1) * 128],
                psT[:])

        def emit_outproj(m, split=False):
            stage = stpool.tile([128, E], FP16, tag="st", name=f"st{m}")
            for j in range(2):
                t = psM.tile([128, 512], F32, tag="mi", name=f"op{m}_{j}")
                for pair in range(2):
                    nc.tensor.matmul(
                        t[:],
                        oT2_sb[:, pair, m * 128:(m + 1) * 128],
                        wo2_sb[:, pair, j * 512:(j + 1) * 512],
                        start=(pair == 0), stop=(pair == 1),
                    )
                if split and j == 1:
                    nc.scalar.copy(stage[:, 512:1024], t[:])
                else:
                    nc.vector.tensor_copy(stage[:, j * 512:(j + 1) * 512], t[:])
            nc.gpsimd.dma_start(out[m * 128:(m + 1) * 128, :], stage[:])

        # ---- prefix: K/Q projections, nch-split so pair0 lands fast ----
        # psA (4 banks) coexists with psS (4 banks); pair0's K tiles project
        # and copy out first, Q-qc0 goes through psS slots, and the first
        # scores fire while pair1's K projection finishes in the background.
        pts = {}
        cur = []
        vdone = 0
        with tc.tile_pool(name="psA", bufs=4, space="PSUM") as psA:
            kt0 = [psA.tile([128, 512], F32, tag="mm", name=f"pk0_{m}")
                   for m in range(4)]
            kt1 = [psA.tile([128, 512], F32, tag="mm", name=f"pk1_{m}")
                   for m in range(4)]
            for e in range(NE):
                for m in range(4):
                    nc.tensor.matmul(
                        kt0[m][:], wk_sb[:, e, 0:128],
                        xk[e][:, m * 512:(m + 1) * 512],
                        start=(e == 0), stop=(e == NE - 1))
            for m in range(4):
                nc.vector.tensor_copy(
                    kT_sb[0][:, m * 512:(m + 1) * 512], kt0[m][:])
            for e in range(5):   # fills the PE while the xq0 DMAs land
                for m in range(4):
                    nc.tensor.matmul(
                        kt1[m][:], wk_sb[:, e, 128:256],
                        xk[e][:, m * 512:(m + 1) * 512],
                        start=(e == 0), stop=False)
            for nch in range(2):
                t = psS.tile([128, 512], F32, tag="ss", name=f"pq0_{nch}")
                for e in range(NE):
                    nc.tensor.matmul(
                        t[:], wq_sb[:, e, nch * 128:(nch + 1) * 128],
                        xq[0][e][:], start=(e == 0), stop=(e == NE - 1))
                nc.vector.tensor_copy(qT_sb[nch][:, 0:512], t[:])
            for k in range(3):   # first scores; K-pair1 e5..e7 in their shadow
                cur.append(emit_scores_exp(0, 0, k))
                for m in range(4):
                    nc.tensor.matmul(
                        kt1[m][:], wk_sb[:, 5 + k, 128:256],
                        xk[5 + k][:, m * 512:(m + 1) * 512],
                        start=False, stop=(5 + k == NE - 1))
            for m in range(4):
                nc.vector.tensor_copy(
                    kT_sb[1][:, m * 512:(m + 1) * 512], kt1[m][:])
            for k in range(3, 6):
                cur.append(emit_scores_exp(0, 0, k))
        psOV = ctx.enter_context(tc.tile_pool(name="psOV", bufs=1, space="PSUM"))
        psM = ctx.enter_context(tc.tile_pool(name="psM", bufs=2, space="PSUM"))

        for k in range(6, NK):
            cur.append(emit_scores_exp(0, 0, k))
            if k >= 12:  # xv DMAs have landed by the time PE reaches these
                emit_vproj(vdone)
                vdone += 1
        pts[(0, 0)] = cur
        emit_vproj(vdone)
        vdone += 1
        emit_qproj(1, 0)
        emit_qproj(1, 1)

        seq = [(qc, pair) for qc in range(NQC) for pair in range(2)]
        trans_pending = None   # (qc, pair, nTs) awaiting transpose weave
        for i, (qc, pair) in enumerate(seq):
            nxt = seq[i + 1] if i + 1 < len(seq) else None
            ps_o = psOV.tile([128, 1024], F32, tag="ov", name=f"ov{qc}_{pair}")
            ovv = ps_o.rearrange("p (h q c) -> p h q c", h=2, q=4)
            cur_pts = pts.pop((qc, pair))
            nxt_pts = [] if nxt else None
            op_ms = list(range((qc - 1) * 4, qc * 4)) if (pair == 0 and qc >= 1) else []
            for k in range(NK):
                if nxt:
                    nxt_pts.append(emit_scores_exp(nxt[0], nxt[1], k))
                if trans_pending and k < 4:
                    tqc, tpair, tnTs = trans_pending
                    emit_transpose(tqc, tpair, k, tnTs[k])
                    if k == 3:
                        trans_pending = None
                if vdone < NK:  # remaining V-proj tiles, just-in-time
                    emit_vproj(vdone)
                    vdone += 1
                emit_pv(pair, k, cur_pts[k], ovv)
                if op_ms and k % 4 == 3:
                    emit_outproj(op_ms[k // 4])
                if pair == 0 and 1 <= qc < NQC - 1 and k in (8, 12):
                    # Q projection for the next qc, before its scores appear
                    emit_qproj(qc + 1, (k - 8) // 4)
            if nxt:
                pts[nxt] = nxt_pts
            if nxt:
                nTs = emit_norm(qc, pair, ovv)
                trans_pending = (qc, pair, nTs)
            else:
                # tail: pipeline norm -> transpose -> out-proj per q-tile,
                # splitting elementwise work across DVE and the now-idle Act
                # ((3,0)'s transposes were already woven into this window)
                nTs = emit_norm(qc, pair, ovv, split=True)
                for qt in range(4):
                    emit_transpose(qc, pair, qt, nTs[qt], pool=psS)
                    emit_outproj((NQC - 1) * 4 + qt, split=True)

    return nc


_NC_CACHE = {}


def _get_nc():
    if "nc" not in _NC_CACHE:
        _NC_CACHE["nc"] = build()
    return _NC_CACHE["nc"]


def _shard_inputs(query, key, value, Wq, Wk, Wv, Wo):
    """Host-side sharding + layout prep: core c = (batch c//4, head-group c%4)."""
    f16 = np.float16
    xT = []
    for b in range(B):
        xT.append((
            np.ascontiguousarray(query[b].T).astype(f16),
            np.ascontiguousarray(key[b].T).astype(f16),
            np.ascontiguousarray(value[b].T).astype(f16),
        ))
    wT = []
    for g in range(4):
        gc = slice(g * 256, (g + 1) * 256)
        wo_g = Wo[:, gc].T.astype(f16)            # [256, E]
        woT2 = np.ascontiguousarray(
            wo_g.reshape(2, 128, E).transpose(1, 0, 2).reshape(128, 2 * E))
        wT.append((
            np.ascontiguousarray(Wq[gc].T).astype(f16),
            np.ascontiguousarray(Wk[gc].T).astype(f16),
            np.ascontiguousarray(Wv[gc].T).astype(f16),
            woT2,
        ))
    ident = np.eye(128, dtype=f16)
    in_maps = []
    for c in range(NCORES):
        b, g = c // 4, c % 4
        qT, kT, vT = xT[b]
        wq, wk, wv, wo2 = wT[g]
        in_maps.append({
            "xqT": qT, "xkT": kT, "xvT": vT,
            "wqT": wq, "wkT": wk, "wvT": wv, "woT2": wo2,
            "ident": ident,
        })
    return in_maps


def kernel(query, key, value, Wq, Wk, Wv, Wo):
    query = np.asarray(query, dtype=np.float32)
    key = np.asarray(key, dtype=np.float32)
    value = np.asarray(value, dtype=np.float32)
    Wq = np.asarray(Wq, dtype=np.float32)
    Wk = np.asarray(Wk, dtype=np.float32)
    Wv = np.asarray(Wv, dtype=np.float32)
    Wo = np.asarray(Wo, dtype=np.float32)

    nc = _get_nc()
    in_maps = _shard_inputs(query, key, value, Wq, Wk, Wv, Wo)
    res = run_bass_kernel_spmd(nc, in_maps, core_ids=list(range(NCORES)))

    out = np.zeros((B, S, E), dtype=np.float32)
    for c in range(NCORES):
        out[c // 4] += res.results[c]["out"].astype(np.float32)
    return out


# revision 34
# speedup vs baseline: 1.3903x; 1.0098x over previous
"""Trainium2 Bass kernel for nn_MultiHeadAttention (B=2, S=2048, E=1024, H=16).

Sharding: 8 NeuronCores = data-parallel over the 2 batches x tensor-parallel
over the 16 heads in 4 groups of 4 heads (Wq/Wk/Wv split column-wise, Wo
row-wise).  Each core computes a full-[S, E] partial of its batch's output;
the host sums the 4 head-group partials per batch.

Per-core device algorithm (S.T orientation feeds a flipped P@V):
  Q.T/K.T[n, s] = (wT chunk).T @ xT chunk       e/m-outer projections
  S.T_h[k, q]   = (K_h.T chunk).T @ Q_h.T       row-packed head pairs (d=64)
  P.T           = exp(S.T / 8)                  fp16, one ACT op per (k, pair)
  O[q, d|sum]   = (P.T chunk).T @ [V_h | 1]     FLIPPED: P.T [128k,128q] is the
                                                stationary, [V|ones] [128k,65]
                                                moving -> 65-col outputs, with
                                                the softmax denominator landing
                                                in column 64 (per-partition!)
  O_n           = O * recip(col 64)             one DVE tensor_scalar per tile
  O.T           = PE-transpose(O_n)             53ns/tile, restores [d, q]
  out[m, :]     = sum_pair (oT2 chunk).T @ woT2 contract-128 output projection

The flip + contract-128 out-proj cut PE matmul time ~27%; exp on the
Activation engine (~133us) becomes the critical resource, so the instruction
stream is interleaved to keep it saturated: each (qc, pair) "window" weaves
the NEXT window's scores+exp with THIS window's PV, plus V/Q-projection,
out-projection, and transpose filler work sized to the Act-engine pace.

dtypes: HBM traffic fp16 (in+out); scores fp32r; P/V/O/Wo fp16; accum fp32.
"""

import numpy as np
from contextlib import ExitStack

import ml_dtypes

import concourse.bass as bass
import concourse.mybir as mybir
import concourse.tile as tile
from concourse.tile import ScopedClock
from concourse.bass_utils import run_bass_kernel_spmd

# ---------------------------------------------------------------------------
# Workarounds for the walrus build on this stack, which rejects more than ONE
# semaphore wait per instruction ("Too many sync wait commands").
# ---------------------------------------------------------------------------
_orig_commit_instruction = tile.TileContext._commit_instruction


def _commit_instruction(self, inst, lazy_reg_writes=True):
    si = getattr(inst, "sync_info", None)
    if si is not None and si.on_wait and len(si.on_wait) > 1:
        waits = list(si.on_wait)
        for w in waits[:-1]:
            nop = mybir.InstNoOp(
                name=self.nc.get_next_instruction_name(),
                ins=[], outs=[], engine=inst.engine,
            )
            nop.bass_nofuse = True
            nop.sync_info = mybir.SyncInfo(on_wait=[w], on_update=[])
            _orig_commit_instruction(self, nop, lazy_reg_writes=False)
        inst.sync_info = mybir.SyncInfo(
            on_wait=[waits[-1]], on_update=list(si.on_update or [])
        )
    return _orig_commit_instruction(self, inst, lazy_reg_writes)


def _drain_and_barrier(self, tick_clock, wait_clock):
    nc = self.nc
    drain_inst = nc.sync.drain()
    wait_clock.add_sem_waits(
        drain_inst.ins, ScopedClock({None: tick_clock.global_clock})
    )
    si = drain_inst.ins.sync_info
    waits = list(si.on_wait) if si and si.on_wait else []
    if len(waits) > 1:
        drain_inst.ins.sync_info = mybir.SyncInfo(
            on_wait=waits[:1], on_update=list(si.on_update or [])
        )
        for w in waits[1:]:
            extra = nc.sync.drain()
            esi = extra.ins.sync_info
            extra.ins.sync_info = mybir.SyncInfo(
                on_wait=[w],
                on_update=list(esi.on_update or []) if esi else [],
            )
    nc.all_engine_barrier()
    assert self.sems is not None
    popped = nc._tile_sem_poison_stack.pop()
    assert popped is self._sem_poison
    nc.clear_and_free_semaphores(list(self.sems.allocated().values()))
    nc.all_engine_barrier()


def _apply_tilefix():
    tile.TileContext._commit_instruction = _commit_instruction
    tile.TileContext._drain_and_barrier = _drain_and_barrier


_apply_tilefix()

# ---------------------------------------------------------------------------
# Problem constants (hardcoded)
# ---------------------------------------------------------------------------
B, S, E, H = 2, 2048, 1024, 16
HC, D = 4, 64              # heads per core, head dim
NCORES = 8
NE = E // 128              # 8  e-chunks (projection contraction)
NK = S // 128              # 16 k-chunks
NQC = S // 512             # 4  512-wide q windows
NM = S // 128              # 16 output row chunks

F32 = mybir.dt.float32
F32R = mybir.dt.float32r
FP16 = mybir.dt.float16
EXP = mybir.ActivationFunctionType.Exp
MULT = mybir.AluOpType.mult


def build(ptbufs=18, xdt=FP16, qkdt=mybir.dt.float32r):
    nc = bass.Bass()
    xqT = nc.dram_tensor("xqT", [E, S], xdt, kind="ExternalInput")
    xkT = nc.dram_tensor("xkT", [E, S], xdt, kind="ExternalInput")
    xvT = nc.dram_tensor("xvT", [E, S], xdt, kind="ExternalInput")
    wqT = nc.dram_tensor("wqT", [E, 256], xdt, kind="ExternalInput")
    wkT = nc.dram_tensor("wkT", [E, 256], xdt, kind="ExternalInput")
    wvT = nc.dram_tensor("wvT", [E, 256], xdt, kind="ExternalInput")
    woT2 = nc.dram_tensor("woT2", [128, 2 * E], FP16, kind="ExternalInput")
    ident = nc.dram_tensor("ident", [128, 128], FP16, kind="ExternalInput")
    out = nc.dram_tensor("out", [S, E], FP16, kind="ExternalOutput")

    with tile.TileContext(nc) as tc, ExitStack() as ctx:
        consts = ctx.enter_context(tc.tile_pool(name="consts", bufs=1))
        wpool = ctx.enter_context(tc.tile_pool(name="w", bufs=1))
        actpool = ctx.enter_context(tc.tile_pool(name="acts", bufs=1))
        xkvpool = ctx.enter_context(tc.tile_pool(name="xkv", bufs=10))
        xqpool = ctx.enter_context(tc.tile_pool(name="xq", bufs=32))

        # preload the exp table before the hot loop
        dummy = consts.tile([1, 8], F32)
        nc.vector.memset(dummy[:], 0.0)
        nc.scalar.activation(dummy[:], dummy[:], EXP)

        wq_sb = wpool.tile([128, NE, 256], xdt)
        wk_sb = wpool.tile([128, NE, 256], xdt)
        wv_sb = wpool.tile([128, NE, 256], xdt)
        wo2_sb = wpool.tile([128, 2, E], FP16)
        id_sb = wpool.tile([128, 128], FP16)

        # per-pair tensors: tile-granular dependency tracking means a read
        # waits on every writer emitted so far, so pair0's scores must not
        # share a tile with pair1's (later-copied) projections
        qT_sb = [actpool.tile([128, S], qkdt, name=f"qT{p}") for p in range(2)]
        kT_sb = [actpool.tile([128, S], qkdt, name=f"kT{p}") for p in range(2)]
        v_sb = actpool.tile([128, NK, HC, 65], FP16)   # [s%128, k, h, V_h|one]
        oT2_sb = actpool.tile([128, 2, S], FP16)       # [(2 heads x d), pair, q]

        # ---- DMA program: ordered by first-use ----
        nc.sync.dma_start(wk_sb[:], wkT.rearrange("(ec p) n -> p ec n", p=128))
        xk = []
        for e in range(NE):
            t = xkvpool.tile([128, S], xdt, tag="xc", name=f"xk{e}")
            nc.sync.dma_start(t[:], xkT[e * 128:(e + 1) * 128, :])
            xk.append(t)
        nc.sync.dma_start(wq_sb[:], wqT.rearrange("(ec p) n -> p ec n", p=128))
        # xq arrives in per-qc column slices so qc0 lands early
        xq = [[None] * NE for _ in range(NQC)]
        for qc in [0]:
            for e in range(NE):
                t = xqpool.tile([128, 512], xdt, tag="xq", name=f"xq{qc}_{e}")
                nc.sync.dma_start(
                    t[:], xqT[e * 128:(e + 1) * 128, qc * 512:(qc + 1) * 512])
                xq[qc][e] = t
        nc.sync.dma_start(wv_sb[:], wvT.rearrange("(ec p) n -> p ec n", p=128))
        xv = []
        for e in range(NE):
            t = xkvpool.tile([128, S], xdt, tag="xc", name=f"xv{e}")
            nc.sync.dma_start(t[:], xvT[e * 128:(e + 1) * 128, :])
            xv.append(t)
        nc.sync.dma_start(
            wo2_sb[:], woT2.rearrange("p (two e) -> p two e", two=2))
        nc.sync.dma_start(id_sb[:], ident[0:128, :])
        for qc in range(1, NQC):
            for e in range(NE):
                t = xqpool.tile([128, 512], xdt, tag="xq", name=f"xq{qc}_{e}")
                nc.sync.dma_start(
                    t[:], xqT[e * 128:(e + 1) * 128, qc * 512:(qc + 1) * 512])
                xq[qc][e] = t

        # ones column for the fused rowsum (PV moving operand col 64)
        nc.vector.memset(v_sb[:, :, :, 64:65], 1.0)

        # ---- steady-state pools (psS first: it coexists with prefix psA) ----
        psS = ctx.enter_context(tc.tile_pool(name="psS", bufs=2, space="PSUM"))
        ppool = ctx.enter_context(tc.tile_pool(name="pT", bufs=ptbufs))
        npool = ctx.enter_context(tc.tile_pool(name="nT", bufs=6))
        rpool = ctx.enter_context(tc.tile_pool(name="rt", bufs=4))
        stpool = ctx.enter_context(tc.tile_pool(name="stage", bufs=4))

        def emit_scores_exp(qc, pair, k):
            qs = slice(qc * 512, (qc + 1) * 512)
            ks = slice(k * 128, (k + 1) * 128)
            ps_s = psS.tile([128, 1024], F32, tag="ss", name=f"ss{qc}_{pair}_{k}")
            nc.tensor.matmul(ps_s[:, 0:512],
                             kT_sb[pair][0:64, ks], qT_sb[pair][0:64, qs],
                             start=True, stop=True)
            nc.tensor.matmul(ps_s[:, 512:1024],
                             kT_sb[pair][64:128, ks], qT_sb[pair][64:128, qs],
                             start=True, stop=True)
            pT = ppool.tile([128, 1024], FP16, tag="pt", name=f"pt{qc}_{pair}_{k}")
            nc.scalar.activation(pT[:], ps_s[:], EXP, scale=0.125)
            return pT

        def emit_pv(pair, k, pT, ovv, qts=(0, 1, 2, 3)):
            # ovv: ps_o viewed as [128, qt, h2, 128]; O in cols 0:64, sum col
            # 64. qt-major so qt 0-1 fill PSUM bank0 and qt 2-3 bank1 (the
            # last window is drained per qt-half). PSUM zero regions are 2KB:
            # start=True pending-zeroes the WHOLE bank, so only the first
            # region per bank may start the group; the other regions' k==0
            # writes land on pending-zero bytes and write-through (hardware
            # lazy-zero semantics).
            for qt in qts:
                for h2 in range(2):
                    h = pair * 2 + h2
                    nc.tensor.matmul(
                        ovv[:, qt, h2, 0:65],
                        pT[:, h2 * 512 + qt * 128:h2 * 512 + (qt + 1) * 128],
                        v_sb[:, k, h, 0:65],
                        start=(k == 0 and h2 == 0 and qt in (0, 2)),
                        stop=(k == NK - 1),
                        skip_group_check=True,
                    )

        def emit_vproj(k):
            t = psM.tile([128, 512], F32, tag="mi", name=f"vp{k}")
            for e in range(NE):
                nc.tensor.matmul(
                    t[:, 0:256],
                    xv[e][:, k * 128:(k + 1) * 128],
                    wv_sb[:, e, :],
                    start=(e == 0), stop=(e == NE - 1),
                )
            nc.vector.tensor_copy(
                v_sb[:, k, :, 0:64],
                t[:, 0:256].rearrange("p (h c) -> p h c", h=HC))

        def emit_qproj(qc, nch):
            t = psM.tile([128, 512], F32, tag="mi", name=f"qp{qc}_{nch}")
            for e in range(NE):
                nc.tensor.matmul(
                    t[:],
                    wq_sb[:, e, nch * 128:(nch + 1) * 128],
                    xq[qc][e][:],
                    start=(e == 0), stop=(e == NE - 1),
                )
            nc.vector.tensor_copy(qT_sb[nch][:, qc * 512:(qc + 1) * 512], t[:])

        def emit_norm(qc, pair, ovv, qts=(0, 1, 2, 3), split=False):
            # reciprocal of the fused rowsums (col 64 of each 128-col region)
            n = len(qts)
            rt = rpool.tile([128, 2 * n], F32, tag="rt",
                            name=f"rt{qc}_{pair}_{qts[0]}")
            nc.vector.reciprocal(
                rt[:].rearrange("p (q h) -> p q h", q=n),
                ovv[:, qts[0]:qts[0] + n, :, 64])
            nTs = {}
            for i, qt in enumerate(qts):
                nT = npool.tile([128, 128], FP16, tag="nt",
                                name=f"nt{qc}_{pair}_{qt}")
                for h2 in range(2):
                    if split and h2 == 1:
                        # tail only: Act engine is idle, halve the DVE chain
                        nc.scalar.mul(nT[:, 64:128], ovv[:, qt, 1, 0:64],
                                      rt[:, 2 * i + 1:2 * i + 2])
                    else:
                        nc.vector.tensor_scalar(
                            nT[:, h2 * 64:(h2 + 1) * 64],
                            ovv[:, qt, h2, 0:64],
                            rt[:, 2 * i + h2:2 * i + h2 + 1],
                            None, MULT)
                nTs[qt] = nT
            return nTs

        def emit_transpose(qc, pair, qt, nT, pool=None):
            # pool=psS in the tail: scores are done, its slots are idle, and
            # keeping transposes out of the psM ring stops them serializing
            # behind out-proj tiles awaiting their stage copies.
            pool = pool or psM
            tag = "ss" if pool is psS else "mi"
            psT = pool.tile([128, 128], FP16, tag=tag, name=f"tp{qc}_{pair}_{qt}")
            nc.tensor.transpose(psT[:], nT[:], id_sb[:])
            nc.vector.tensor_copy(
                oT2_sb[:, pair, qc * 512 + qt * 128:qc * 512 + (qt + 1) * 128],
                psT[:])

        def emit_outproj(m, split=False, pool=None):
            pool = pool or psM
            tag = {id(psS): "ss", id(psOV): "ov"}.get(id(pool), "mi")
            stage = stpool.tile([128, E], FP16, tag="st", name=f"st{m}")
            for j in range(2):
                t = pool.tile([128, 512], F32, tag=tag, name=f"op{m}_{j}")
                for pair in range(2):
                    nc.tensor.matmul(
                        t[:],
                        oT2_sb[:, pair, m * 128:(m + 1) * 128],
                        wo2_sb[:, pair, j * 512:(j + 1) * 512],
                        start=(pair == 0), stop=(pair == 1),
                    )
                if split and j == 1:
                    nc.scalar.copy(stage[:, 512:1024], t[:])
                else:
                    nc.vector.tensor_copy(stage[:, j * 512:(j + 1) * 512], t[:])
            if split:
                nc.sync.dma_start(out[m * 128:(m + 1) * 128, :], stage[:])
            else:
                nc.gpsimd.dma_start(out[m * 128:(m + 1) * 128, :], stage[:])

        # ---- prefix: K/Q projections, nch-split so pair0 lands fast ----
        # psA (4 banks) coexists with psS (4 banks); pair0's K tiles project
        # and copy out first, Q-qc0 goes through psS slots, and the first
        # scores fire while pair1's K projection finishes in the background.
        pts = {}
        cur = []
        vdone = 0
        with tc.tile_pool(name="psA", bufs=4, space="PSUM") as psA:
            kt0 = [psA.tile([128, 512], F32, tag="mm", name=f"pk0_{m}")
                   for m in range(4)]
            kt1 = [psA.tile([128, 512], F32, tag="mm", name=f"pk1_{m}")
                   for m in range(4)]
            for e in range(NE):
                for m in range(4):
                    nc.tensor.matmul(
                        kt0[m][:], wk_sb[:, e, 0:128],
                        xk[e][:, m * 512:(m + 1) * 512],
                        start=(e == 0), stop=(e == NE - 1))
            for m in range(4):
                nc.vector.tensor_copy(
                    kT_sb[0][:, m * 512:(m + 1) * 512], kt0[m][:])
            # Q-qc0 pair0 chases the xq0 DMAs immediately after K-pair0 (the
            # PE stream is in-order, so nothing may sit between them); all of
            # pair1's projection work weaves into the first scores' shadows.
            tq0 = psS.tile([128, 512], F32, tag="ss", name="pq0_0")
            # tq1 must NOT take a psS ring slot: its reader (the qT1 copy)
            # lands after the early scores, which would deadlock their
            # tile allocations. The psA ring's kt0[0] slot frees early.
            tq1 = psA.tile([128, 512], F32, tag="mm", name="pq0_1")
            for e in range(NE):
                nc.tensor.matmul(
                    tq0[:], wq_sb[:, e, 0:128], xq[0][e][:],
                    start=(e == 0), stop=(e == NE - 1))
            nc.vector.tensor_copy(qT_sb[0][:, 0:512], tq0[:])
            for k in range(6):
                cur.append(emit_scores_exp(0, 0, k))
                if k < 4:     # K-pair1, two e-chunks per score shadow
                    for e in (2 * k, 2 * k + 1):
                        for m in range(4):
                            nc.tensor.matmul(
                                kt1[m][:], wk_sb[:, e, 128:256],
                                xk[e][:, m * 512:(m + 1) * 512],
                                start=(e == 0), stop=(e == NE - 1))
                elif k < 6:   # then Q-pair1, four e-chunks per shadow
                    for e in range(4 * (k - 4), 4 * (k - 3)):
                        nc.tensor.matmul(
                            tq1[:], wq_sb[:, e, 128:256], xq[0][e][:],
                            start=(e == 0), stop=(e == NE - 1))
            for m in range(4):
                nc.vector.tensor_copy(
                    kT_sb[1][:, m * 512:(m + 1) * 512], kt1[m][:])
            for k in range(6, 8):
                cur.append(emit_scores_exp(0, 0, k))
                if k == 6:
                    nc.vector.tensor_copy(qT_sb[1][:, 0:512], tq1[:])
        psOV = ctx.enter_context(tc.tile_pool(name="psOV", bufs=1, space="PSUM"))
        psM = ctx.enter_context(tc.tile_pool(name="psM", bufs=2, space="PSUM"))

        for k in range(8, NK):
            cur.append(emit_scores_exp(0, 0, k))
            if k >= 9 and vdone < 5:   # xv DMAs have landed by these shadows
                emit_vproj(vdone)
                vdone += 1
            elif k == 14:
                emit_qproj(1, 0)
            elif k == 15:
                emit_qproj(1, 1)
        pts[(0, 0)] = cur

        seq = [(qc, pair) for qc in range(NQC) for pair in range(2)]
        trans_pending = None   # (qc, pair, nTs) awaiting transpose weave
        for i, (qc, pair) in enumerate(seq):
            nxt = seq[i + 1] if i + 1 < len(seq) else None
            ps_o = psOV.tile([128, 1024], F32, tag="ov", name=f"ov{qc}_{pair}")
            ovv = ps_o.rearrange("p (q h c) -> p q h c", q=4, h=2)
            cur_pts = pts.pop((qc, pair))
            nxt_pts = [] if nxt else None
            op_ms = list(range((qc - 1) * 4, qc * 4)) if (pair == 0 and qc >= 1) else []
            last_qts = (0, 1, 2, 3) if nxt else (0, 1)
            for k in range(NK):
                if nxt:
                    nxt_pts.append(emit_scores_exp(nxt[0], nxt[1], k))
                if trans_pending and k < 4:
                    tqc, tpair, tnTs = trans_pending
                    emit_transpose(tqc, tpair, k, tnTs[k])
                    if k == 3:
                        trans_pending = None
                if vdone < NK:  # remaining V-proj tiles, just-in-time
                    emit_vproj(vdone)
                    vdone += 1
                emit_pv(pair, k, cur_pts[k], ovv, qts=last_qts)
                if op_ms and k % 4 == 3:
                    emit_outproj(op_ms[k // 4])
                if pair == 0 and 1 <= qc < NQC - 1 and k in (8, 12):
                    # Q projection for the next qc, before its scores appear
                    emit_qproj(qc + 1, (k - 8) // 4)
            if nxt:
                pts[nxt] = nxt_pts
                nTs = emit_norm(qc, pair, ovv)
                trans_pending = (qc, pair, nTs)
            else:
                # tail: the window was PV'd for q-tiles 0-1 only; drain that
                # half (norm -> transpose -> out-proj, elementwise work split
                # across DVE and the idle Act engine) while q-tiles 2-3 run
                # their PV pass. Both norms reach the DVE queue before any
                # stage copy: DVE executes in order, so a stage copy emitted
                # early would delay the second half's whole chain.
                mb = (NQC - 1) * 4
                nTsA = emit_norm(qc, pair, ovv, qts=(0, 1), split=True)
                for k in range(NK):
                    emit_pv(pair, k, cur_pts[k], ovv, qts=(2, 3))
                    if k == 4:
                        emit_transpose(qc, pair, 0, nTsA[0], pool=psS)
                    elif k == 6:
                        emit_transpose(qc, pair, 1, nTsA[1], pool=psS)
                nTsB = emit_norm(qc, pair, ovv, qts=(2, 3), split=True)
                # distribute the 4 m-tiles across 3 PSUM rings (psOV is free
                # once normB has read it) so their chains run concurrently
                emit_outproj(mb + 0, split=True, pool=psM)
                emit_outproj(mb + 1, split=True, pool=psS)
                emit_transpose(qc, pair, 2, nTsB[2], pool=psS)
                emit_transpose(qc, pair, 3, nTsB[3], pool=psS)
                emit_outproj(mb + 2, split=True, pool=psOV)
                emit_outproj(mb + 3, split=True, pool=psM)

    return nc


_NC_CACHE = {}


def _get_nc():
    if "nc" not in _NC_CACHE:
        _NC_CACHE["nc"] = build()
    return _NC_CACHE["nc"]


def _shard_inputs(query, key, value, Wq, Wk, Wv, Wo):
    """Host-side sharding + layout prep: core c = (batch c//4, head-group c%4)."""
    f16 = np.float16
    xT = []
    for b in range(B):
        xT.append((
            np.ascontiguousarray(query[b].T).astype(f16),
            np.ascontiguousarray(key[b].T).astype(f16),
            np.ascontiguousarray(value[b].T).astype(f16),
        ))
    wT = []
    for g in range(4):
        gc = slice(g * 256, (g + 1) * 256)
        wo_g = Wo[:, gc].T.astype(f16)            # [256, E]
        woT2 = np.ascontiguousarray(
            wo_g.reshape(2, 128, E).transpose(1, 0, 2).reshape(128, 2 * E))
        wT.append((
            np.ascontiguousarray(Wq[gc].T).astype(f16),
            np.ascontiguousarray(Wk[gc].T).astype(f16),
            np.ascontiguousarray(Wv[gc].T).astype(f16),
            woT2,
        ))
    ident = np.eye(128, dtype=f16)
    in_maps = []
    for c in range(NCORES):
        b, g = c // 4, c % 4
        qT, kT, vT = xT[b]
        wq, wk, wv, wo2 = wT[g]
        in_maps.append({
            "xqT": qT, "xkT": kT, "xvT": vT,
            "wqT": wq, "wkT": wk, "wvT": wv, "woT2": wo2,
            "ident": ident,
        })
    return in_maps


def kernel(query, key, value, Wq, Wk, Wv, Wo):
    query = np.asarray(query, dtype=np.float32)
    key = np.asarray(key, dtype=np.float32)
    value = np.asarray(value, dtype=np.float32)
    Wq = np.asarray(Wq, dtype=np.float32)
    Wk = np.asarray(Wk, dtype=np.float32)
    Wv = np.asarray(Wv, dtype=np.float32)
    Wo = np.asarray(Wo, dtype=np.float32)

    nc = _get_nc()
    in_maps = _shard_inputs(query, key, value, Wq, Wk, Wv, Wo)
    res = run_bass_kernel_spmd(nc, in_maps, core_ids=list(range(NCORES)))

    out = np.zeros((B, S, E), dtype=np.float32)
    for c in range(NCORES):
        out[c // 4] += res.results[c]["out"].astype(np.float32)
    return out


# revision 44
# speedup vs baseline: 1.3958x; 1.0039x over previous
"""Trainium2 Bass kernel for nn_MultiHeadAttention (B=2, S=2048, E=1024, H=16).

Sharding: 8 NeuronCores = data-parallel over the 2 batches x tensor-parallel
over the 16 heads in 4 groups of 4 heads (Wq/Wk/Wv split column-wise, Wo
row-wise).  Each core computes a full-[S, E] partial of its batch's output;
the host sums the 4 head-group partials per batch.

Per-core device algorithm (S.T orientation feeds a flipped P@V):
  Q.T/K.T[n, s] = (wT chunk).T @ xT chunk       e/m-outer projections
  S.T_h[k, q]   = (K_h.T chunk).T @ Q_h.T       row-packed head pairs (d=64)
  P.T           = exp(S.T / 8)                  fp16, one ACT op per (k, pair)
  O[q, d|sum]   = (P.T chunk).T @ [V_h | 1]     FLIPPED: P.T [128k,128q] is the
                                                stationary, [V|ones] [128k,65]
                                                moving -> 65-col outputs, with
                                                the softmax denominator landing
                                                in column 64 (per-partition!)
  O_n           = O * recip(col 64)             one DVE tensor_scalar per tile
  O.T           = PE-transpose(O_n)             53ns/tile, restores [d, q]
  out[m, :]     = sum_pair (oT2 chunk).T @ woT2 contract-128 output projection

The flip + contract-128 out-proj cut PE matmul time ~27%; exp on the
Activation engine (~133us) becomes the critical resource, so the instruction
stream is interleaved to keep it saturated: each (qc, pair) "window" weaves
the NEXT window's scores+exp with THIS window's PV, plus V/Q-projection,
out-projection, and transpose filler work sized to the Act-engine pace.

dtypes: HBM traffic fp16 (in+out); scores fp32r; P/V/O/Wo fp16; accum fp32.
"""

import numpy as np
from contextlib import ExitStack

import ml_dtypes

import concourse.bass as bass
import concourse.mybir as mybir
import concourse.tile as tile
from concourse.tile import ScopedClock
from concourse.bass_utils import run_bass_kernel_spmd

# ---------------------------------------------------------------------------
# Workarounds for the walrus build on this stack, which rejects more than ONE
# semaphore wait per instruction ("Too many sync wait commands").
# ---------------------------------------------------------------------------
_orig_commit_instruction = tile.TileContext._commit_instruction


def _commit_instruction(self, inst, lazy_reg_writes=True):
    si = getattr(inst, "sync_info", None)
    if si is not None and si.on_wait and len(si.on_wait) > 1:
        waits = list(si.on_wait)
        for w in waits[:-1]:
            nop = mybir.InstNoOp(
                name=self.nc.get_next_instruction_name(),
                ins=[], outs=[], engine=inst.engine,
            )
            nop.bass_nofuse = True
            nop.sync_info = mybir.SyncInfo(on_wait=[w], on_update=[])
            _orig_commit_instruction(self, nop, lazy_reg_writes=False)
        inst.sync_info = mybir.SyncInfo(
            on_wait=[waits[-1]], on_update=list(si.on_update or [])
        )
    return _orig_commit_instruction(self, inst, lazy_reg_writes)


def _drain_and_barrier(self, tick_clock, wait_clock):
    nc = self.nc
    drain_inst = nc.sync.drain()
    wait_clock.add_sem_waits(
        drain_inst.ins, ScopedClock({None: tick_clock.global_clock})
    )
    si = drain_inst.ins.sync_info
    waits = list(si.on_wait) if si and si.on_wait else []
    if len(waits) > 1:
        drain_inst.ins.sync_info = mybir.SyncInfo(
            on_wait=waits[:1], on_update=list(si.on_update or [])
        )
        for w in waits[1:]:
            extra = nc.sync.drain()
            esi = extra.ins.sync_info
            extra.ins.sync_info = mybir.SyncInfo(
                on_wait=[w],
                on_update=list(esi.on_update or []) if esi else [],
            )
    nc.all_engine_barrier()
    assert self.sems is not None
    popped = nc._tile_sem_poison_stack.pop()
    assert popped is self._sem_poison
    nc.clear_and_free_semaphores(list(self.sems.allocated().values()))
    nc.all_engine_barrier()


def _apply_tilefix():
    tile.TileContext._commit_instruction = _commit_instruction
    tile.TileContext._drain_and_barrier = _drain_and_barrier


_apply_tilefix()

# ---------------------------------------------------------------------------
# Problem constants (hardcoded)
# ---------------------------------------------------------------------------
B, S, E, H = 2, 2048, 1024, 16
HC, D = 4, 64              # heads per core, head dim
NCORES = 8
NE = E // 128              # 8  e-chunks (projection contraction)
NK = S // 128              # 16 k-chunks
NQC = S // 512             # 4  512-wide q windows
NM = S // 128              # 16 output row chunks

F32 = mybir.dt.float32
F32R = mybir.dt.float32r
FP16 = mybir.dt.float16
EXP = mybir.ActivationFunctionType.Exp
MULT = mybir.AluOpType.mult


def build(ptbufs=20, xdt=FP16, qkdt=mybir.dt.float32r):
    nc = bass.Bass()
    xqT = nc.dram_tensor("xqT", [E, S], xdt, kind="ExternalInput")
    xkT = nc.dram_tensor("xkT", [E, S], xdt, kind="ExternalInput")
    xvT = nc.dram_tensor("xvT", [E, S], xdt, kind="ExternalInput")
    wqT = nc.dram_tensor("wqT", [E, 256], xdt, kind="ExternalInput")
    wkT = nc.dram_tensor("wkT", [E, 256], xdt, kind="ExternalInput")
    wvT = nc.dram_tensor("wvT", [E, 256], xdt, kind="ExternalInput")
    woT2 = nc.dram_tensor("woT2", [128, 2 * E], FP16, kind="ExternalInput")
    ident = nc.dram_tensor("ident", [128, 128], FP16, kind="ExternalInput")
    out = nc.dram_tensor("out", [S, E], FP16, kind="ExternalOutput")

    with tile.TileContext(nc) as tc, ExitStack() as ctx:
        consts = ctx.enter_context(tc.tile_pool(name="consts", bufs=1))
        wpool = ctx.enter_context(tc.tile_pool(name="w", bufs=1))
        actpool = ctx.enter_context(tc.tile_pool(name="acts", bufs=1))
        xkvpool = ctx.enter_context(tc.tile_pool(name="xkv", bufs=10))
        xqpool = ctx.enter_context(tc.tile_pool(name="xq", bufs=32))

        # preload the exp table before the hot loop
        dummy = consts.tile([1, 8], F32)
        nc.vector.memset(dummy[:], 0.0)
        nc.scalar.activation(dummy[:], dummy[:], EXP)

        wq_sb = wpool.tile([128, NE, 256], xdt)
        wk_sb = wpool.tile([128, NE, 256], xdt)
        wv_sb = wpool.tile([128, NE, 256], xdt)
        wo2_sb = wpool.tile([128, 2, E], FP16)
        id_sb = wpool.tile([128, 128], FP16)

        # per-pair tensors: tile-granular dependency tracking means a read
        # waits on every writer emitted so far, so pair0's scores must not
        # share a tile with pair1's (later-copied) projections
        qT_sb = [actpool.tile([128, S], qkdt, name=f"qT{p}") for p in range(2)]
        kT_sb = [actpool.tile([128, S], qkdt, name=f"kT{p}") for p in range(2)]
        v_sb = actpool.tile([128, NK, HC, 65], FP16)   # [s%128, k, h, V_h|one]
        oT2_sb = actpool.tile([128, 2, S], FP16)       # [(2 heads x d), pair, q]

        # ---- DMA program: ordered by first-use ----
        nc.sync.dma_start(wk_sb[:], wkT.rearrange("(ec p) n -> p ec n", p=128))
        xk = []
        for e in range(NE):
            t = xkvpool.tile([128, S], xdt, tag="xc", name=f"xk{e}")
            nc.sync.dma_start(t[:], xkT[e * 128:(e + 1) * 128, :])
            xk.append(t)
        nc.sync.dma_start(wq_sb[:], wqT.rearrange("(ec p) n -> p ec n", p=128))
        # xq arrives in per-qc column slices so qc0 lands early
        xq = [[None] * NE for _ in range(NQC)]
        for qc in [0]:
            for e in range(NE):
                t = xqpool.tile([128, 512], xdt, tag="xq", name=f"xq{qc}_{e}")
                nc.sync.dma_start(
                    t[:], xqT[e * 128:(e + 1) * 128, qc * 512:(qc + 1) * 512])
                xq[qc][e] = t
        nc.sync.dma_start(wv_sb[:], wvT.rearrange("(ec p) n -> p ec n", p=128))
        xv = []
        for e in range(NE):
            t = xkvpool.tile([128, S], xdt, tag="xc", name=f"xv{e}")
            nc.sync.dma_start(t[:], xvT[e * 128:(e + 1) * 128, :])
            xv.append(t)
        for qc in range(1, NQC):
            if qc == 2:
                # wo2/ident ride between xq1 and xq2 (first use ~55us); they
                # must not delay xq1, which gates the qc1 Q-projection
                nc.sync.dma_start(
                    wo2_sb[:], woT2.rearrange("p (two e) -> p two e", two=2))
                nc.sync.dma_start(id_sb[:], ident[0:128, :])
            for e in range(NE):
                t = xqpool.tile([128, 512], xdt, tag="xq", name=f"xq{qc}_{e}")
                nc.sync.dma_start(
                    t[:], xqT[e * 128:(e + 1) * 128, qc * 512:(qc + 1) * 512])
                xq[qc][e] = t

        # ones column for the fused rowsum (PV moving operand col 64)
        nc.vector.memset(v_sb[:, :, :, 64:65], 1.0)

        # ---- steady-state pools (psS first: it coexists with prefix psA) ----
        psS = ctx.enter_context(tc.tile_pool(name="psS", bufs=2, space="PSUM"))
        ppool = ctx.enter_context(tc.tile_pool(name="pT", bufs=ptbufs))
        npool = ctx.enter_context(tc.tile_pool(name="nT", bufs=6))
        rpool = ctx.enter_context(tc.tile_pool(name="rt", bufs=4))
        stpool = ctx.enter_context(tc.tile_pool(name="stage", bufs=4))

        def emit_scores_exp(qc, pair, k):
            qs = slice(qc * 512, (qc + 1) * 512)
            ks = slice(k * 128, (k + 1) * 128)
            ps_s = psS.tile([128, 1024], F32, tag="ss", name=f"ss{qc}_{pair}_{k}")
            nc.tensor.matmul(ps_s[:, 0:512],
                             kT_sb[pair][0:64, ks], qT_sb[pair][0:64, qs],
                             start=True, stop=True)
            nc.tensor.matmul(ps_s[:, 512:1024],
                             kT_sb[pair][64:128, ks], qT_sb[pair][64:128, qs],
                             start=True, stop=True)
            pT = ppool.tile([128, 1024], FP16, tag="pt", name=f"pt{qc}_{pair}_{k}")
            nc.scalar.activation(pT[:], ps_s[:], EXP, scale=0.125)
            return pT

        def emit_pv(pair, k, pT, ovv, qts=(0, 1, 2, 3)):
            # ovv: ps_o viewed as [128, qt, h2, 128]; O in cols 0:64, sum col
            # 64. qt-major so qt 0-1 fill PSUM bank0 and qt 2-3 bank1 (the
            # last window is drained per qt-half). PSUM zero regions are 2KB:
            # start=True pending-zeroes the WHOLE bank, so only the first
            # region per bank may start the group; the other regions' k==0
            # writes land on pending-zero bytes and write-through (hardware
            # lazy-zero semantics).
            for qt in qts:
                for h2 in range(2):
                    h = pair * 2 + h2
                    nc.tensor.matmul(
                        ovv[:, qt, h2, 0:65],
                        pT[:, h2 * 512 + qt * 128:h2 * 512 + (qt + 1) * 128],
                        v_sb[:, k, h, 0:65],
                        start=(k == 0 and h2 == 0 and qt in (0, 2)),
                        stop=(k == NK - 1),
                        skip_group_check=True,
                    )

        def emit_vproj(k):
            t = psM.tile([128, 512], F32, tag="mi", name=f"vp{k}")
            for e in range(NE):
                nc.tensor.matmul(
                    t[:, 0:256],
                    xv[e][:, k * 128:(k + 1) * 128],
                    wv_sb[:, e, :],
                    start=(e == 0), stop=(e == NE - 1),
                )
            nc.vector.tensor_copy(
                v_sb[:, k, :, 0:64],
                t[:, 0:256].rearrange("p (h c) -> p h c", h=HC))

        def emit_qproj(qc, nch):
            t = psM.tile([128, 512], F32, tag="mi", name=f"qp{qc}_{nch}")
            for e in range(NE):
                nc.tensor.matmul(
                    t[:],
                    wq_sb[:, e, nch * 128:(nch + 1) * 128],
                    xq[qc][e][:],
                    start=(e == 0), stop=(e == NE - 1),
                )
            nc.vector.tensor_copy(qT_sb[nch][:, qc * 512:(qc + 1) * 512], t[:])

        def emit_norm(qc, pair, ovv, qts=(0, 1, 2, 3), split=False):
            # reciprocal of the fused rowsums (col 64 of each 128-col region)
            n = len(qts)
            rt = rpool.tile([128, 2 * n], F32, tag="rt",
                            name=f"rt{qc}_{pair}_{qts[0]}")
            nc.vector.reciprocal(
                rt[:].rearrange("p (q h) -> p q h", q=n),
                ovv[:, qts[0]:qts[0] + n, :, 64])
            nTs = {}
            for i, qt in enumerate(qts):
                nT = npool.tile([128, 128], FP16, tag="nt",
                                name=f"nt{qc}_{pair}_{qt}")
                for h2 in range(2):
                    if split and h2 == 1:
                        # tail only: Act engine is idle, halve the DVE chain
                        nc.scalar.mul(nT[:, 64:128], ovv[:, qt, 1, 0:64],
                                      rt[:, 2 * i + 1:2 * i + 2])
                    else:
                        nc.vector.tensor_scalar(
                            nT[:, h2 * 64:(h2 + 1) * 64],
                            ovv[:, qt, h2, 0:64],
                            rt[:, 2 * i + h2:2 * i + h2 + 1],
                            None, MULT)
                nTs[qt] = nT
            return nTs

        def emit_transpose(qc, pair, qt, nT, pool=None):
            # pool=psS in the tail: scores are done, its slots are idle, and
            # keeping transposes out of the psM ring stops them serializing
            # behind out-proj tiles awaiting their stage copies.
            pool = pool or psM
            tag = {id(psS): "ss", id(psOV): "ov"}.get(id(pool), "mi")
            psT = pool.tile([128, 128], FP16, tag=tag, name=f"tp{qc}_{pair}_{qt}")
            nc.tensor.transpose(psT[:], nT[:], id_sb[:])
            nc.vector.tensor_copy(
                oT2_sb[:, pair, qc * 512 + qt * 128:qc * 512 + (qt + 1) * 128],
                psT[:])

        def emit_outproj(m, split=False, pool=None):
            pool = pool or psM
            tag = {id(psS): "ss", id(psOV): "ov"}.get(id(pool), "mi")
            stage = stpool.tile([128, E], FP16, tag="st", name=f"st{m}")
            for j in range(2):
                t = pool.tile([128, 512], F32, tag=tag, name=f"op{m}_{j}")
                for pair in range(2):
                    nc.tensor.matmul(
                        t[:],
                        oT2_sb[:, pair, m * 128:(m + 1) * 128],
                        wo2_sb[:, pair, j * 512:(j + 1) * 512],
                        start=(pair == 0), stop=(pair == 1),
                    )
                if split and j == 1:
                    nc.scalar.copy(stage[:, 512:1024], t[:])
                else:
                    nc.vector.tensor_copy(stage[:, j * 512:(j + 1) * 512], t[:])
            if split:
                nc.sync.dma_start(out[m * 128:(m + 1) * 128, :], stage[:])
            else:
                nc.gpsimd.dma_start(out[m * 128:(m + 1) * 128, :], stage[:])

        # ---- prefix: K/Q projections, nch-split so pair0 lands fast ----
        # psA (4 banks) coexists with psS (4 banks); pair0's K tiles project
        # and copy out first, Q-qc0 goes through psS slots, and the first
        # scores fire while pair1's K projection finishes in the background.
        pts = {}
        cur = []
        vdone = 0
        with tc.tile_pool(name="psA", bufs=4, space="PSUM") as psA:
            kt0 = [psA.tile([128, 512], F32, tag="mm", name=f"pk0_{m}")
                   for m in range(4)]
            kt1 = [psA.tile([128, 512], F32, tag="mm", name=f"pk1_{m}")
                   for m in range(4)]
            for e in range(NE):
                for m in range(4):
                    nc.tensor.matmul(
                        kt0[m][:], wk_sb[:, e, 0:128],
                        xk[e][:, m * 512:(m + 1) * 512],
                        start=(e == 0), stop=(e == NE - 1))
            for m in range(4):
                nc.vector.tensor_copy(
                    kT_sb[0][:, m * 512:(m + 1) * 512], kt0[m][:])
            # Q-qc0 pair0 chases the xq0 DMAs immediately after K-pair0 (the
            # PE stream is in-order, so nothing may sit between them); all of
            # pair1's projection work weaves into the first scores' shadows.
            tq0 = psS.tile([128, 512], F32, tag="ss", name="pq0_0")
            # tq1 must NOT take a psS ring slot: its reader (the qT1 copy)
            # lands after the early scores, which would deadlock their
            # tile allocations. The psA ring's kt0[0] slot frees early.
            tq1 = psA.tile([128, 512], F32, tag="mm", name="pq0_1")
            for e in range(NE):
                nc.tensor.matmul(
                    tq0[:], wq_sb[:, e, 0:128], xq[0][e][:],
                    start=(e == 0), stop=(e == NE - 1))
            nc.vector.tensor_copy(qT_sb[0][:, 0:512], tq0[:])
            for k in range(6):
                cur.append(emit_scores_exp(0, 0, k))
                if k < 4:     # K-pair1, two e-chunks per score shadow
                    for e in (2 * k, 2 * k + 1):
                        for m in range(4):
                            nc.tensor.matmul(
                                kt1[m][:], wk_sb[:, e, 128:256],
                                xk[e][:, m * 512:(m + 1) * 512],
                                start=(e == 0), stop=(e == NE - 1))
                elif k < 6:   # then Q-pair1, four e-chunks per shadow
                    for e in range(4 * (k - 4), 4 * (k - 3)):
                        nc.tensor.matmul(
                            tq1[:], wq_sb[:, e, 128:256], xq[0][e][:],
                            start=(e == 0), stop=(e == NE - 1))
            for m in range(4):
                nc.vector.tensor_copy(
                    kT_sb[1][:, m * 512:(m + 1) * 512], kt1[m][:])
            for k in range(6, 8):
                cur.append(emit_scores_exp(0, 0, k))
                if k == 6:
                    nc.vector.tensor_copy(qT_sb[1][:, 0:512], tq1[:])
        psOV = ctx.enter_context(tc.tile_pool(name="psOV", bufs=1, space="PSUM"))
        psM = ctx.enter_context(tc.tile_pool(name="psM", bufs=2, space="PSUM"))

        for k in range(8, NK):
            cur.append(emit_scores_exp(0, 0, k))
            if k >= 9 and vdone < 5:   # xv DMAs have landed by these shadows
                emit_vproj(vdone)
                vdone += 1
            elif k == 14:
                emit_qproj(1, 0)
            elif k == 15:
                emit_qproj(1, 1)
        pts[(0, 0)] = cur

        seq = [(qc, pair) for qc in range(NQC) for pair in range(2)]
        trans_pending = None   # (qc, pair, nTs) awaiting transpose weave
        for i, (qc, pair) in enumerate(seq):
            nxt = seq[i + 1] if i + 1 < len(seq) else None
            ps_o = psOV.tile([128, 1024], F32, tag="ov", name=f"ov{qc}_{pair}")
            ovv = ps_o.rearrange("p (q h c) -> p q h c", q=4, h=2)
            cur_pts = pts.pop((qc, pair))
            nxt_pts = [] if nxt else None
            op_ms = list(range((qc - 1) * 4, qc * 4)) if (pair == 0 and qc >= 1) else []
            last_qts = (0, 1, 2, 3)
            for k in range(NK):
                if nxt:
                    nxt_pts.append(emit_scores_exp(nxt[0], nxt[1], k))
                if trans_pending and k < 4:
                    tqc, tpair, tnTs = trans_pending
                    emit_transpose(tqc, tpair, k, tnTs[k])
                    if k == 3:
                        trans_pending = None
                if vdone < NK:  # remaining V-proj tiles, just-in-time
                    emit_vproj(vdone)
                    vdone += 1
                emit_pv(pair, k, cur_pts[k], ovv, qts=last_qts)
                if op_ms and k % 4 == 3:
                    emit_outproj(op_ms[k // 4])
                if pair == 0 and 1 <= qc < NQC - 1 and k in (8, 12):
                    # Q projection for the next qc, before its scores appear
                    emit_qproj(qc + 1, (k - 8) // 4)
            if nxt:
                pts[nxt] = nxt_pts
                nTs = emit_norm(qc, pair, ovv)
                trans_pending = (qc, pair, nTs)
            else:
                # tail drain: norm (split across DVE + the idle Act engine),
                # then per-q-tile transpose + out-proj chains distributed
                # over all three PSUM rings so they pipeline
                mb = (NQC - 1) * 4
                nTs = emit_norm(qc, pair, ovv, split=True)
                emit_transpose(qc, pair, 0, nTs[0], pool=psS)
                emit_outproj(mb + 0, split=True, pool=psM)
                emit_transpose(qc, pair, 1, nTs[1], pool=psS)
                emit_outproj(mb + 1, split=True, pool=psS)
                emit_transpose(qc, pair, 2, nTs[2], pool=psOV)
                emit_outproj(mb + 2, split=True, pool=psM)
                emit_transpose(qc, pair, 3, nTs[3], pool=psOV)
                emit_outproj(mb + 3, split=True, pool=psS)

    return nc


_NC_CACHE = {}


def _get_nc():
    if "nc" not in _NC_CACHE:
        _NC_CACHE["nc"] = build()
    return _NC_CACHE["nc"]


def _shard_inputs(query, key, value, Wq, Wk, Wv, Wo):
    """Host-side sharding + layout prep: core c = (batch c//4, head-group c%4)."""
    f16 = np.float16
    xT = []
    for b in range(B):
        xT.append((
            np.ascontiguousarray(query[b].T).astype(f16),
            np.ascontiguousarray(key[b].T).astype(f16),
            np.ascontiguousarray(value[b].T).astype(f16),
        ))
    wT = []
    for g in range(4):
        gc = slice(g * 256, (g + 1) * 256)
        wo_g = Wo[:, gc].T.astype(f16)            # [256, E]
        woT2 = np.ascontiguousarray(
            wo_g.reshape(2, 128, E).transpose(1, 0, 2).reshape(128, 2 * E))
        wT.append((
            np.ascontiguousarray(Wq[gc].T).astype(f16),
            np.ascontiguousarray(Wk[gc].T).astype(f16),
            np.ascontiguousarray(Wv[gc].T).astype(f16),
            woT2,
        ))
    ident = np.eye(128, dtype=f16)
    in_maps = []
    for c in range(NCORES):
        b, g = c // 4, c % 4
        qT, kT, vT = xT[b]
        wq, wk, wv, wo2 = wT[g]
        in_maps.append({
            "xqT": qT, "xkT": kT, "xvT": vT,
            "wqT": wq, "wkT": wk, "wvT": wv, "woT2": wo2,
            "ident": ident,
        })
    return in_maps


def kernel(query, key, value, Wq, Wk, Wv, Wo):
    query = np.asarray(query, dtype=np.float32)
    key = np.asarray(key, dtype=np.float32)
    value = np.asarray(value, dtype=np.float32)
    Wq = np.asarray(Wq, dtype=np.float32)
    Wk = np.asarray(Wk, dtype=np.float32)
    Wv = np.asarray(Wv, dtype=np.float32)
    Wo = np.asarray(Wo, dtype=np.float32)

    nc = _get_nc()
    in_maps = _shard_inputs(query, key, value, Wq, Wk, Wv, Wo)
    res = run_bass_kernel_spmd(nc, in_maps, core_ids=list(range(NCORES)))

    out = np.zeros((B, S, E), dtype=np.float32)
    for c in range(NCORES):
        out[c // 4] += res.results[c]["out"].astype(np.float32)
    return out


# revision 47
# speedup vs baseline: 1.4002x; 1.0032x over previous
"""Trainium2 Bass kernel for nn_MultiHeadAttention (B=2, S=2048, E=1024, H=16).

Sharding: 8 NeuronCores = data-parallel over the 2 batches x tensor-parallel
over the 16 heads in 4 groups of 4 heads (Wq/Wk/Wv split column-wise, Wo
row-wise).  Each core computes a full-[S, E] partial of its batch's output;
the host sums the 4 head-group partials per batch.

Per-core device algorithm (S.T orientation feeds a flipped P@V):
  Q.T/K.T[n, s] = (wT chunk).T @ xT chunk       e/m-outer projections
  S.T_h[k, q]   = (K_h.T chunk).T @ Q_h.T       row-packed head pairs (d=64)
  P.T           = exp(S.T / 8)                  fp16, one ACT op per (k, pair)
  O[q, d|sum]   = (P.T chunk).T @ [V_h | 1]     FLIPPED: P.T [128k,128q] is the
                                                stationary, [V|ones] [128k,65]
                                                moving -> 65-col outputs, with
                                                the softmax denominator landing
                                                in column 64 (per-partition!)
  O_n           = O * recip(col 64)             one DVE tensor_scalar per tile
  O.T           = PE-transpose(O_n)             53ns/tile, restores [d, q]
  out[m, :]     = sum_pair (oT2 chunk).T @ woT2 contract-128 output projection

The flip + contract-128 out-proj cut PE matmul time ~27%; exp on the
Activation engine (~133us) becomes the critical resource, so the instruction
stream is interleaved to keep it saturated: each (qc, pair) "window" weaves
the NEXT window's scores+exp with THIS window's PV, plus V/Q-projection,
out-projection, and transpose filler work sized to the Act-engine pace.

dtypes: HBM traffic fp16 (in+out); scores fp32r; P/V/O/Wo fp16; accum fp32.
"""

import numpy as np
from contextlib import ExitStack

import ml_dtypes

import concourse.bass as bass
import concourse.mybir as mybir
import concourse.tile as tile
from concourse.tile import ScopedClock
from concourse.bass_utils import run_bass_kernel_spmd

# ---------------------------------------------------------------------------
# Workarounds for the walrus build on this stack, which rejects more than ONE
# semaphore wait per instruction ("Too many sync wait commands").
# ---------------------------------------------------------------------------
_orig_commit_instruction = tile.TileContext._commit_instruction


def _commit_instruction(self, inst, lazy_reg_writes=True):
    si = getattr(inst, "sync_info", None)
    if si is not None and si.on_wait and len(si.on_wait) > 1:
        waits = list(si.on_wait)
        for w in waits[:-1]:
            nop = mybir.InstNoOp(
                name=self.nc.get_next_instruction_name(),
                ins=[], outs=[], engine=inst.engine,
            )
            nop.bass_nofuse = True
            nop.sync_info = mybir.SyncInfo(on_wait=[w], on_update=[])
            _orig_commit_instruction(self, nop, lazy_reg_writes=False)
        inst.sync_info = mybir.SyncInfo(
            on_wait=[waits[-1]], on_update=list(si.on_update or [])
        )
    return _orig_commit_instruction(self, inst, lazy_reg_writes)


def _drain_and_barrier(self, tick_clock, wait_clock):
    nc = self.nc
    drain_inst = nc.sync.drain()
    wait_clock.add_sem_waits(
        drain_inst.ins, ScopedClock({None: tick_clock.global_clock})
    )
    si = drain_inst.ins.sync_info
    waits = list(si.on_wait) if si and si.on_wait else []
    if len(waits) > 1:
        drain_inst.ins.sync_info = mybir.SyncInfo(
            on_wait=waits[:1], on_update=list(si.on_update or [])
        )
        for w in waits[1:]:
            extra = nc.sync.drain()
            esi = extra.ins.sync_info
            extra.ins.sync_info = mybir.SyncInfo(
                on_wait=[w],
                on_update=list(esi.on_update or []) if esi else [],
            )
    nc.all_engine_barrier()
    assert self.sems is not None
    popped = nc._tile_sem_poison_stack.pop()
    assert popped is self._sem_poison
    nc.clear_and_free_semaphores(list(self.sems.allocated().values()))
    nc.all_engine_barrier()


def _apply_tilefix():
    tile.TileContext._commit_instruction = _commit_instruction
    tile.TileContext._drain_and_barrier = _drain_and_barrier


_apply_tilefix()

# ---------------------------------------------------------------------------
# Problem constants (hardcoded)
# ---------------------------------------------------------------------------
B, S, E, H = 2, 2048, 1024, 16
HC, D = 4, 64              # heads per core, head dim
NCORES = 8
NE = E // 128              # 8  e-chunks (projection contraction)
NK = S // 128              # 16 k-chunks
NQC = S // 512             # 4  512-wide q windows
NM = S // 128              # 16 output row chunks

F32 = mybir.dt.float32
F32R = mybir.dt.float32r
FP16 = mybir.dt.float16
EXP = mybir.ActivationFunctionType.Exp
MULT = mybir.AluOpType.mult


def build(ptbufs=20, xdt=FP16, qkdt=mybir.dt.float32r):
    nc = bass.Bass()
    xqT = nc.dram_tensor("xqT", [E, S], xdt, kind="ExternalInput")
    xkT = nc.dram_tensor("xkT", [E, S], xdt, kind="ExternalInput")
    xvT = nc.dram_tensor("xvT", [E, S], xdt, kind="ExternalInput")
    wqT = nc.dram_tensor("wqT", [E, 256], xdt, kind="ExternalInput")
    wkT = nc.dram_tensor("wkT", [E, 256], xdt, kind="ExternalInput")
    wvT = nc.dram_tensor("wvT", [E, 256], xdt, kind="ExternalInput")
    woT2 = nc.dram_tensor("woT2", [128, 2 * E], FP16, kind="ExternalInput")
    ident = nc.dram_tensor("ident", [128, 128], FP16, kind="ExternalInput")
    out = nc.dram_tensor("out", [S, E], FP16, kind="ExternalOutput")

    with tile.TileContext(nc) as tc, ExitStack() as ctx:
        consts = ctx.enter_context(tc.tile_pool(name="consts", bufs=1))
        wpool = ctx.enter_context(tc.tile_pool(name="w", bufs=1))
        actpool = ctx.enter_context(tc.tile_pool(name="acts", bufs=1))
        xkvpool = ctx.enter_context(tc.tile_pool(name="xkv", bufs=10))
        xqpool = ctx.enter_context(tc.tile_pool(name="xq", bufs=32))

        # preload the exp table before the hot loop
        dummy = consts.tile([1, 8], F32)
        nc.vector.memset(dummy[:], 0.0)
        nc.scalar.activation(dummy[:], dummy[:], EXP)

        wq_sb = wpool.tile([128, NE, 256], xdt)
        wk_sb = wpool.tile([128, NE, 256], xdt)
        wv_sb = wpool.tile([128, NE, 256], xdt)
        wo2_sb = wpool.tile([128, 2, E], FP16)
        id_sb = wpool.tile([128, 128], FP16)

        # per-pair tensors: tile-granular dependency tracking means a read
        # waits on every writer emitted so far, so pair0's scores must not
        # share a tile with pair1's (later-copied) projections
        qT_sb = [actpool.tile([128, S], qkdt, name=f"qT{p}") for p in range(2)]
        kT_sb = [actpool.tile([128, S], qkdt, name=f"kT{p}") for p in range(2)]
        v_sb = actpool.tile([128, NK, HC, 65], FP16)   # [s%128, k, h, V_h|one]
        oT2_sb = actpool.tile([128, 2, S], FP16)       # [(2 heads x d), pair, q]

        # ---- DMA program: ordered by first-use ----
        nc.sync.dma_start(wk_sb[:], wkT.rearrange("(ec p) n -> p ec n", p=128))
        xk = []
        for e in range(NE):
            t = xkvpool.tile([128, S], xdt, tag="xc", name=f"xk{e}")
            nc.sync.dma_start(t[:], xkT[e * 128:(e + 1) * 128, :])
            xk.append(t)
        nc.sync.dma_start(wq_sb[:], wqT.rearrange("(ec p) n -> p ec n", p=128))
        # xq arrives in per-qc column slices so qc0 lands early
        xq = [[None] * NE for _ in range(NQC)]
        for qc in [0]:
            for e in range(NE):
                t = xqpool.tile([128, 512], xdt, tag="xq", name=f"xq{qc}_{e}")
                nc.sync.dma_start(
                    t[:], xqT[e * 128:(e + 1) * 128, qc * 512:(qc + 1) * 512])
                xq[qc][e] = t
        nc.sync.dma_start(wv_sb[:], wvT.rearrange("(ec p) n -> p ec n", p=128))
        xv = []
        for e in range(NE):
            t = xkvpool.tile([128, S], xdt, tag="xc", name=f"xv{e}")
            nc.sync.dma_start(t[:], xvT[e * 128:(e + 1) * 128, :])
            xv.append(t)
        for qc in range(1, NQC):
            if qc == 2:
                # wo2/ident ride between xq1 and xq2 (first use ~55us); they
                # must not delay xq1, which gates the qc1 Q-projection
                nc.sync.dma_start(
                    wo2_sb[:], woT2.rearrange("p (two e) -> p two e", two=2))
                nc.sync.dma_start(id_sb[:], ident[0:128, :])
            for e in range(NE):
                t = xqpool.tile([128, 512], xdt, tag="xq", name=f"xq{qc}_{e}")
                nc.sync.dma_start(
                    t[:], xqT[e * 128:(e + 1) * 128, qc * 512:(qc + 1) * 512])
                xq[qc][e] = t

        # ones column for the fused rowsum (PV moving operand col 64)
        nc.vector.memset(v_sb[:, :, :, 64:65], 1.0)

        # ---- steady-state pools (psS first: it coexists with prefix psA) ----
        psS = ctx.enter_context(tc.tile_pool(name="psS", bufs=2, space="PSUM"))
        ppool = ctx.enter_context(tc.tile_pool(name="pT", bufs=ptbufs))
        npool = ctx.enter_context(tc.tile_pool(name="nT", bufs=6))
        rpool = ctx.enter_context(tc.tile_pool(name="rt", bufs=4))
        stpool = ctx.enter_context(tc.tile_pool(name="stage", bufs=4))

        def emit_scores_exp(qc, pair, k):
            qs = slice(qc * 512, (qc + 1) * 512)
            ks = slice(k * 128, (k + 1) * 128)
            ps_s = psS.tile([128, 1024], F32, tag="ss", name=f"ss{qc}_{pair}_{k}")
            nc.tensor.matmul(ps_s[:, 0:512],
                             kT_sb[pair][0:64, ks], qT_sb[pair][0:64, qs],
                             start=True, stop=True)
            nc.tensor.matmul(ps_s[:, 512:1024],
                             kT_sb[pair][64:128, ks], qT_sb[pair][64:128, qs],
                             start=True, stop=True)
            pT = ppool.tile([128, 1024], FP16, tag="pt", name=f"pt{qc}_{pair}_{k}")
            nc.scalar.activation(pT[:], ps_s[:], EXP, scale=0.125)
            return pT

        def emit_pv(pair, k, pT, ovv, qts=(0, 1, 2, 3)):
            # ovv: ps_o viewed as [128, qt, h2, 128]; O in cols 0:64, sum col
            # 64. qt-major so qt 0-1 fill PSUM bank0 and qt 2-3 bank1 (the
            # last window is drained per qt-half). PSUM zero regions are 2KB:
            # start=True pending-zeroes the WHOLE bank, so only the first
            # region per bank may start the group; the other regions' k==0
            # writes land on pending-zero bytes and write-through (hardware
            # lazy-zero semantics).
            for qt in qts:
                for h2 in range(2):
                    h = pair * 2 + h2
                    nc.tensor.matmul(
                        ovv[:, qt, h2, 0:65],
                        pT[:, h2 * 512 + qt * 128:h2 * 512 + (qt + 1) * 128],
                        v_sb[:, k, h, 0:65],
                        start=(k == 0 and h2 == 0 and qt in (0, 2)),
                        stop=(k == NK - 1),
                        skip_group_check=True,
                    )

        def emit_vproj(k):
            t = psM.tile([128, 512], F32, tag="mi", name=f"vp{k}")
            for e in range(NE):
                nc.tensor.matmul(
                    t[:, 0:256],
                    xv[e][:, k * 128:(k + 1) * 128],
                    wv_sb[:, e, :],
                    start=(e == 0), stop=(e == NE - 1),
                )
            nc.vector.tensor_copy(
                v_sb[:, k, :, 0:64],
                t[:, 0:256].rearrange("p (h c) -> p h c", h=HC))

        def emit_qproj(qc, nch):
            t = psM.tile([128, 512], F32, tag="mi", name=f"qp{qc}_{nch}")
            for e in range(NE):
                nc.tensor.matmul(
                    t[:],
                    wq_sb[:, e, nch * 128:(nch + 1) * 128],
                    xq[qc][e][:],
                    start=(e == 0), stop=(e == NE - 1),
                )
            nc.vector.tensor_copy(qT_sb[nch][:, qc * 512:(qc + 1) * 512], t[:])

        def emit_norm(qc, pair, ovv, qts=(0, 1, 2, 3), split=False):
            # reciprocal of the fused rowsums (col 64 of each 128-col region)
            n = len(qts)
            rt = rpool.tile([128, 2 * n], F32, tag="rt",
                            name=f"rt{qc}_{pair}_{qts[0]}")
            nc.vector.reciprocal(
                rt[:].rearrange("p (q h) -> p q h", q=n),
                ovv[:, qts[0]:qts[0] + n, :, 64])
            nTs = {}
            for i, qt in enumerate(qts):
                nT = npool.tile([128, 128], FP16, tag="nt",
                                name=f"nt{qc}_{pair}_{qt}")
                for h2 in range(2):
                    if split and h2 == 1:
                        # tail only: Act engine is idle, halve the DVE chain
                        nc.scalar.mul(nT[:, 64:128], ovv[:, qt, 1, 0:64],
                                      rt[:, 2 * i + 1:2 * i + 2])
                    else:
                        nc.vector.tensor_scalar(
                            nT[:, h2 * 64:(h2 + 1) * 64],
                            ovv[:, qt, h2, 0:64],
                            rt[:, 2 * i + h2:2 * i + h2 + 1],
                            None, MULT)
                nTs[qt] = nT
            return nTs

        def emit_transpose(qc, pair, qt, nT, pool=None):
            # pool=psS in the tail: scores are done, its slots are idle, and
            # keeping transposes out of the psM ring stops them serializing
            # behind out-proj tiles awaiting their stage copies.
            pool = pool or psM
            tag = {id(psS): "ss", id(psOV): "ov"}.get(id(pool), "mi")
            psT = pool.tile([128, 128], FP16, tag=tag, name=f"tp{qc}_{pair}_{qt}")
            nc.tensor.transpose(psT[:], nT[:], id_sb[:])
            nc.vector.tensor_copy(
                oT2_sb[:, pair, qc * 512 + qt * 128:qc * 512 + (qt + 1) * 128],
                psT[:])

        def emit_outproj(m, split=False, pool=None):
            pool = pool or psM
            tag = {id(psS): "ss", id(psOV): "ov"}.get(id(pool), "mi")
            stage = stpool.tile([128, E], FP16, tag="st", name=f"st{m}")
            for j in range(2):
                t = pool.tile([128, 512], F32, tag=tag, name=f"op{m}_{j}")
                for pair in range(2):
                    nc.tensor.matmul(
                        t[:],
                        oT2_sb[:, pair, m * 128:(m + 1) * 128],
                        wo2_sb[:, pair, j * 512:(j + 1) * 512],
                        start=(pair == 0), stop=(pair == 1),
                    )
                if split and j == 1:
                    nc.scalar.copy(stage[:, 512:1024], t[:])
                else:
                    nc.vector.tensor_copy(stage[:, j * 512:(j + 1) * 512], t[:])
            if split:
                nc.sync.dma_start(out[m * 128:(m + 1) * 128, :], stage[:])
            else:
                nc.gpsimd.dma_start(out[m * 128:(m + 1) * 128, :], stage[:])

        # ---- prefix: K/Q projections, nch-split so pair0 lands fast ----
        # psA (4 banks) coexists with psS (4 banks); pair0's K tiles project
        # and copy out first, Q-qc0 goes through psS slots, and the first
        # scores fire while pair1's K projection finishes in the background.
        pts = {}
        cur = []
        vdone = 0
        with tc.tile_pool(name="psA", bufs=4, space="PSUM") as psA:
            kt0 = [psA.tile([128, 512], F32, tag="mm", name=f"pk0_{m}")
                   for m in range(4)]
            kt1 = [psA.tile([128, 512], F32, tag="mm", name=f"pk1_{m}")
                   for m in range(4)]
            for e in range(NE):
                for m in range(4):
                    nc.tensor.matmul(
                        kt0[m][:], wk_sb[:, e, 0:128],
                        xk[e][:, m * 512:(m + 1) * 512],
                        start=(e == 0), stop=(e == NE - 1))
            for m in range(4):
                nc.vector.tensor_copy(
                    kT_sb[0][:, m * 512:(m + 1) * 512], kt0[m][:])
            # Q-qc0 pair0 chases the xq0 DMAs immediately after K-pair0 (the
            # PE stream is in-order, so nothing may sit between them); all of
            # pair1's projection work weaves into the first scores' shadows.
            tq0 = psS.tile([128, 512], F32, tag="ss", name="pq0_0")
            # tq1 must NOT take a psS ring slot: its reader (the qT1 copy)
            # lands after the early scores, which would deadlock their
            # tile allocations. The psA ring's kt0[0] slot frees early.
            tq1 = psA.tile([128, 512], F32, tag="mm", name="pq0_1")
            for e in range(NE):
                nc.tensor.matmul(
                    tq0[:], wq_sb[:, e, 0:128], xq[0][e][:],
                    start=(e == 0), stop=(e == NE - 1))
            nc.vector.tensor_copy(qT_sb[0][:, 0:512], tq0[:])
            for k in range(6):
                cur.append(emit_scores_exp(0, 0, k))
                if k < 4:     # K-pair1, two e-chunks per score shadow
                    for e in (2 * k, 2 * k + 1):
                        for m in range(4):
                            nc.tensor.matmul(
                                kt1[m][:], wk_sb[:, e, 128:256],
                                xk[e][:, m * 512:(m + 1) * 512],
                                start=(e == 0), stop=(e == NE - 1))
                elif k < 6:   # then Q-pair1, four e-chunks per shadow
                    for e in range(4 * (k - 4), 4 * (k - 3)):
                        nc.tensor.matmul(
                            tq1[:], wq_sb[:, e, 128:256], xq[0][e][:],
                            start=(e == 0), stop=(e == NE - 1))
            for m in range(4):
                nc.vector.tensor_copy(
                    kT_sb[1][:, m * 512:(m + 1) * 512], kt1[m][:])
            for k in range(6, 8):
                cur.append(emit_scores_exp(0, 0, k))
                if k == 6:
                    nc.vector.tensor_copy(qT_sb[1][:, 0:512], tq1[:])
        psOV = ctx.enter_context(tc.tile_pool(name="psOV", bufs=1, space="PSUM"))
        psM = ctx.enter_context(tc.tile_pool(name="psM", bufs=2, space="PSUM"))

        for k in range(8, NK):
            cur.append(emit_scores_exp(0, 0, k))
            if k >= 9 and vdone < 5:   # xv DMAs have landed by these shadows
                emit_vproj(vdone)
                vdone += 1
            elif k == 14:
                emit_qproj(1, 0)
            elif k == 15:
                emit_qproj(1, 1)
        pts[(0, 0)] = cur

        seq = [(qc, pair) for qc in range(NQC) for pair in range(2)]
        trans_pending = None   # (qc, pair, nTs) awaiting transpose weave
        for i, (qc, pair) in enumerate(seq):
            nxt = seq[i + 1] if i + 1 < len(seq) else None
            ps_o = psOV.tile([128, 1024], F32, tag="ov", name=f"ov{qc}_{pair}")
            ovv = ps_o.rearrange("p (q h c) -> p q h c", q=4, h=2)
            cur_pts = pts.pop((qc, pair))
            nxt_pts = [] if nxt else None
            op_ms = list(range((qc - 1) * 4, qc * 4)) if (pair == 0 and qc >= 1) else []
            last_qts = (0, 1, 2, 3)
            for k in range(NK):
                if nxt:
                    nxt_pts.append(emit_scores_exp(nxt[0], nxt[1], k))
                if trans_pending and k < 4:
                    tqc, tpair, tnTs = trans_pending
                    emit_transpose(tqc, tpair, k, tnTs[k])
                    if k == 3:
                        trans_pending = None
                if vdone < NK:  # remaining V-proj tiles, just-in-time
                    emit_vproj(vdone)
                    vdone += 1
                emit_pv(pair, k, cur_pts[k], ovv, qts=last_qts)
                if op_ms and k % 4 == 3:
                    emit_outproj(op_ms[k // 4])
                if pair == 0 and 1 <= qc < NQC - 1 and k in (8, 12):
                    # Q projection for the next qc, before its scores appear
                    emit_qproj(qc + 1, (k - 8) // 4)
            if nxt:
                pts[nxt] = nxt_pts
                nTs = emit_norm(qc, pair, ovv)
                trans_pending = (qc, pair, nTs)
            else:
                # tail drain: norm (split across DVE + the idle Act engine),
                # then per-q-tile transpose + out-proj chains distributed
                # over all three PSUM rings so they pipeline
                mb = (NQC - 1) * 4
                nTs = emit_norm(qc, pair, ovv, split=True)
                emit_transpose(qc, pair, 0, nTs[0], pool=psS)
                emit_transpose(qc, pair, 1, nTs[1], pool=psS)
                emit_transpose(qc, pair, 2, nTs[2], pool=psOV)
                emit_transpose(qc, pair, 3, nTs[3], pool=psOV)
                emit_outproj(mb + 0, split=True, pool=psM)
                emit_outproj(mb + 1, split=True, pool=psS)
                emit_outproj(mb + 2, split=True, pool=psM)
                emit_outproj(mb + 3, split=True, pool=psS)

    return nc


_NC_CACHE = {}


def _get_nc():
    if "nc" not in _NC_CACHE:
        _NC_CACHE["nc"] = build()
    return _NC_CACHE["nc"]


def _shard_inputs(query, key, value, Wq, Wk, Wv, Wo):
    """Host-side sharding + layout prep: core c = (batch c//4, head-group c%4)."""
    f16 = np.float16
    xT = []
    for b in range(B):
        xT.append((
            np.ascontiguousarray(query[b].T).astype(f16),
            np.ascontiguousarray(key[b].T).astype(f16),
            np.ascontiguousarray(value[b].T).astype(f16),
        ))
    wT = []
    for g in range(4):
        gc = slice(g * 256, (g + 1) * 256)
        wo_g = Wo[:, gc].T.astype(f16)            # [256, E]
        woT2 = np.ascontiguousarray(
            wo_g.reshape(2, 128, E).transpose(1, 0, 2).reshape(128, 2 * E))
        wT.append((
            np.ascontiguousarray(Wq[gc].T).astype(f16),
            np.ascontiguousarray(Wk[gc].T).astype(f16),
            np.ascontiguousarray(Wv[gc].T).astype(f16),
            woT2,
        ))
    ident = np.eye(128, dtype=f16)
    in_maps = []
    for c in range(NCORES):
        b, g = c // 4, c % 4
        qT, kT, vT = xT[b]
        wq, wk, wv, wo2 = wT[g]
        in_maps.append({
            "xqT": qT, "xkT": kT, "xvT": vT,
            "wqT": wq, "wkT": wk, "wvT": wv, "woT2": wo2,
            "ident": ident,
        })
    return in_maps


def kernel(query, key, value, Wq, Wk, Wv, Wo):
    query = np.asarray(query, dtype=np.float32)
    key = np.asarray(key, dtype=np.float32)
    value = np.asarray(value, dtype=np.float32)
    Wq = np.asarray(Wq, dtype=np.float32)
    Wk = np.asarray(Wk, dtype=np.float32)
    Wv = np.asarray(Wv, dtype=np.float32)
    Wo = np.asarray(Wo, dtype=np.float32)

    nc = _get_nc()
    in_maps = _shard_inputs(query, key, value, Wq, Wk, Wv, Wo)
    res = run_bass_kernel_spmd(nc, in_maps, core_ids=list(range(NCORES)))

    out = np.zeros((B, S, E), dtype=np.float32)
    for c in range(NCORES):
        out[c // 4] += res.results[c]["out"].astype(np.float32)
    return out


# revision 53
# speedup vs baseline: 1.4054x; 1.0037x over previous
"""Trainium2 Bass kernel for nn_MultiHeadAttention (B=2, S=2048, E=1024, H=16).

Sharding: 8 NeuronCores = data-parallel over the 2 batches x tensor-parallel
over the 16 heads in 4 groups of 4 heads (Wq/Wk/Wv split column-wise, Wo
row-wise).  Each core computes a full-[S, E] partial of its batch's output;
the host sums the 4 head-group partials per batch.

Per-core device algorithm (S.T orientation feeds a flipped P@V):
  Q.T/K.T[n, s] = (wT chunk).T @ xT chunk       e/m-outer projections
  S.T_h[k, q]   = (K_h.T chunk).T @ Q_h.T       row-packed head pairs (d=64)
  P.T           = exp(S.T / 8)                  fp16, one ACT op per (k, pair)
  O[q, d|sum]   = (P.T chunk).T @ [V_h | 1]     FLIPPED: P.T [128k,128q] is the
                                                stationary, [V|ones] [128k,65]
                                                moving -> 65-col outputs, with
                                                the softmax denominator landing
                                                in column 64 (per-partition!)
  O_n           = O * recip(col 64)             one DVE tensor_scalar per tile
  O.T           = PE-transpose(O_n)             53ns/tile, restores [d, q]
  out[m, :]     = sum_pair (oT2 chunk).T @ woT2 contract-128 output projection

The flip + contract-128 out-proj cut PE matmul time ~27%; exp on the
Activation engine (~133us) becomes the critical resource, so the instruction
stream is interleaved to keep it saturated: each (qc, pair) "window" weaves
the NEXT window's scores+exp with THIS window's PV, plus V/Q-projection,
out-projection, and transpose filler work sized to the Act-engine pace.

dtypes: HBM traffic fp16 (in+out); scores fp32r; P/V/O/Wo fp16; accum fp32.
"""

import numpy as np
from contextlib import ExitStack

import ml_dtypes

import concourse.bass as bass
import concourse.mybir as mybir
import concourse.tile as tile
from concourse.tile import ScopedClock
from concourse.bass_utils import run_bass_kernel_spmd

# ---------------------------------------------------------------------------
# Workarounds for the walrus build on this stack, which rejects more than ONE
# semaphore wait per instruction ("Too many sync wait commands").
# ---------------------------------------------------------------------------
_orig_commit_instruction = tile.TileContext._commit_instruction


def _commit_instruction(self, inst, lazy_reg_writes=True):
    si = getattr(inst, "sync_info", None)
    if si is not None and si.on_wait and len(si.on_wait) > 1:
        waits = list(si.on_wait)
        for w in waits[:-1]:
            nop = mybir.InstNoOp(
                name=self.nc.get_next_instruction_name(),
                ins=[], outs=[], engine=inst.engine,
            )
            nop.bass_nofuse = True
            nop.sync_info = mybir.SyncInfo(on_wait=[w], on_update=[])
            _orig_commit_instruction(self, nop, lazy_reg_writes=False)
        inst.sync_info = mybir.SyncInfo(
            on_wait=[waits[-1]], on_update=list(si.on_update or [])
        )
    return _orig_commit_instruction(self, inst, lazy_reg_writes)


def _drain_and_barrier(self, tick_clock, wait_clock):
    nc = self.nc
    drain_inst = nc.sync.drain()
    wait_clock.add_sem_waits(
        drain_inst.ins, ScopedClock({None: tick_clock.global_clock})
    )
    si = drain_inst.ins.sync_info
    waits = list(si.on_wait) if si and si.on_wait else []
    if len(waits) > 1:
        drain_inst.ins.sync_info = mybir.SyncInfo(
            on_wait=waits[:1], on_update=list(si.on_update or [])
        )
        for w in waits[1:]:
            extra = nc.sync.drain()
            esi = extra.ins.sync_info
            extra.ins.sync_info = mybir.SyncInfo(
                on_wait=[w],
                on_update=list(esi.on_update or []) if esi else [],
            )
    nc.all_engine_barrier()
    assert self.sems is not None
    popped = nc._tile_sem_poison_stack.pop()
    assert popped is self._sem_poison
    nc.clear_and_free_semaphores(list(self.sems.allocated().values()))
    nc.all_engine_barrier()


def _apply_tilefix():
    tile.TileContext._commit_instruction = _commit_instruction
    tile.TileContext._drain_and_barrier = _drain_and_barrier


_apply_tilefix()

# ---------------------------------------------------------------------------
# Problem constants (hardcoded)
# ---------------------------------------------------------------------------
B, S, E, H = 2, 2048, 1024, 16
HC, D = 4, 64              # heads per core, head dim
NCORES = 8
NE = E // 128              # 8  e-chunks (projection contraction)
NK = S // 128              # 16 k-chunks
NQC = S // 512             # 4  512-wide q windows
NM = S // 128              # 16 output row chunks

F32 = mybir.dt.float32
F32R = mybir.dt.float32r
FP16 = mybir.dt.float16
EXP = mybir.ActivationFunctionType.Exp
MULT = mybir.AluOpType.mult


def build(ptbufs=20, xdt=FP16, qkdt=mybir.dt.float32r):
    nc = bass.Bass()
    xqT = nc.dram_tensor("xqT", [E, S], xdt, kind="ExternalInput")
    xkT = nc.dram_tensor("xkT", [E, S], xdt, kind="ExternalInput")
    xvT = nc.dram_tensor("xvT", [E, S], xdt, kind="ExternalInput")
    wqT = nc.dram_tensor("wqT", [E, 256], xdt, kind="ExternalInput")
    wkT = nc.dram_tensor("wkT", [E, 256], xdt, kind="ExternalInput")
    wvT = nc.dram_tensor("wvT", [E, 256], xdt, kind="ExternalInput")
    woT2 = nc.dram_tensor("woT2", [128, 2 * E], FP16, kind="ExternalInput")
    ident = nc.dram_tensor("ident", [128, 128], FP16, kind="ExternalInput")
    out = nc.dram_tensor("out", [S, E], FP16, kind="ExternalOutput")

    with tile.TileContext(nc) as tc, ExitStack() as ctx:
        consts = ctx.enter_context(tc.tile_pool(name="consts", bufs=1))
        wpool = ctx.enter_context(tc.tile_pool(name="w", bufs=1))
        actpool = ctx.enter_context(tc.tile_pool(name="acts", bufs=1))
        xkvpool = ctx.enter_context(tc.tile_pool(name="xkv", bufs=10))
        xqpool = ctx.enter_context(tc.tile_pool(name="xq", bufs=32))

        # preload the exp table before the hot loop
        dummy = consts.tile([1, 8], F32)
        nc.vector.memset(dummy[:], 0.0)
        nc.scalar.activation(dummy[:], dummy[:], EXP)

        wq_sb = wpool.tile([128, NE, 256], xdt)
        wk_sb = wpool.tile([128, NE, 256], xdt)
        wv_sb = wpool.tile([128, NE, 256], xdt)
        wo2_sb = wpool.tile([128, 2, E], FP16)
        id_sb = wpool.tile([128, 128], FP16)

        # per-pair tensors: tile-granular dependency tracking means a read
        # waits on every writer emitted so far, so pair0's scores must not
        # share a tile with pair1's (later-copied) projections
        qT_sb = [actpool.tile([128, S], qkdt, name=f"qT{p}") for p in range(2)]
        kT_sb = [actpool.tile([128, S], qkdt, name=f"kT{p}") for p in range(2)]
        v_sb = actpool.tile([128, NK, HC, 65], FP16)   # [s%128, k, h, V_h|one]
        oT2_sb = actpool.tile([128, 2, S], FP16)       # [(2 heads x d), pair, q]

        # ---- DMA program: ordered by first-use ----
        nc.sync.dma_start(wk_sb[:], wkT.rearrange("(ec p) n -> p ec n", p=128))
        nc.sync.dma_start(wq_sb[:], wqT.rearrange("(ec p) n -> p ec n", p=128))
        xk = []
        for e in range(NE):
            t = xkvpool.tile([128, S], xdt, tag="xc", name=f"xk{e}")
            nc.sync.dma_start(t[:], xkT[e * 128:(e + 1) * 128, :])
            xk.append(t)
        # xq arrives in per-qc column slices so qc0 lands early
        xq = [[None] * NE for _ in range(NQC)]
        for qc in [0]:
            for e in range(NE):
                t = xqpool.tile([128, 512], xdt, tag="xq", name=f"xq{qc}_{e}")
                nc.sync.dma_start(
                    t[:], xqT[e * 128:(e + 1) * 128, qc * 512:(qc + 1) * 512])
                xq[qc][e] = t
        nc.sync.dma_start(wv_sb[:], wvT.rearrange("(ec p) n -> p ec n", p=128))
        xv = []
        for e in range(NE):
            t = xkvpool.tile([128, S], xdt, tag="xc", name=f"xv{e}")
            nc.sync.dma_start(t[:], xvT[e * 128:(e + 1) * 128, :])
            xv.append(t)
        for qc in range(1, NQC):
            if qc == 2:
                # wo2/ident ride between xq1 and xq2 (first use ~55us); they
                # must not delay xq1, which gates the qc1 Q-projection
                nc.sync.dma_start(
                    wo2_sb[:], woT2.rearrange("p (two e) -> p two e", two=2))
                nc.sync.dma_start(id_sb[:], ident[0:128, :])
            for e in range(NE):
                t = xqpool.tile([128, 512], xdt, tag="xq", name=f"xq{qc}_{e}")
                nc.sync.dma_start(
                    t[:], xqT[e * 128:(e + 1) * 128, qc * 512:(qc + 1) * 512])
                xq[qc][e] = t

        # ones column for the fused rowsum (PV moving operand col 64)
        nc.vector.memset(v_sb[:, :, :, 64:65], 1.0)

        # ---- steady-state pools (psS first: it coexists with prefix psA) ----
        psS = ctx.enter_context(tc.tile_pool(name="psS", bufs=2, space="PSUM"))
        ppool = ctx.enter_context(tc.tile_pool(name="pT", bufs=ptbufs))
        npool = ctx.enter_context(tc.tile_pool(name="nT", bufs=6))
        rpool = ctx.enter_context(tc.tile_pool(name="rt", bufs=4))
        stpool = ctx.enter_context(tc.tile_pool(name="stage", bufs=4))

        def emit_scores_exp(qc, pair, k):
            qs = slice(qc * 512, (qc + 1) * 512)
            ks = slice(k * 128, (k + 1) * 128)
            ps_s = psS.tile([128, 1024], F32, tag="ss", name=f"ss{qc}_{pair}_{k}")
            nc.tensor.matmul(ps_s[:, 0:512],
                             kT_sb[pair][0:64, ks], qT_sb[pair][0:64, qs],
                             start=True, stop=True)
            nc.tensor.matmul(ps_s[:, 512:1024],
                             kT_sb[pair][64:128, ks], qT_sb[pair][64:128, qs],
                             start=True, stop=True)
            pT = ppool.tile([128, 1024], FP16, tag="pt", name=f"pt{qc}_{pair}_{k}")
            nc.scalar.activation(pT[:], ps_s[:], EXP, scale=0.125)
            return pT

        def emit_pv(pair, k, pT, ovv, qts=(0, 1, 2, 3)):
            # ovv: ps_o viewed as [128, qt, h2, 128]; O in cols 0:64, sum col
            # 64. qt-major so qt 0-1 fill PSUM bank0 and qt 2-3 bank1 (the
            # last window is drained per qt-half). PSUM zero regions are 2KB:
            # start=True pending-zeroes the WHOLE bank, so only the first
            # region per bank may start the group; the other regions' k==0
            # writes land on pending-zero bytes and write-through (hardware
            # lazy-zero semantics).
            for qt in qts:
                for h2 in range(2):
                    h = pair * 2 + h2
                    nc.tensor.matmul(
                        ovv[:, qt, h2, 0:65],
                        pT[:, h2 * 512 + qt * 128:h2 * 512 + (qt + 1) * 128],
                        v_sb[:, k, h, 0:65],
                        start=(k == 0 and h2 == 0 and qt in (0, 2)),
                        stop=(k == NK - 1),
                        skip_group_check=True,
                    )

        def emit_vproj(k):
            t = psM.tile([128, 512], F32, tag="mi", name=f"vp{k}")
            for e in range(NE):
                nc.tensor.matmul(
                    t[:, 0:256],
                    xv[e][:, k * 128:(k + 1) * 128],
                    wv_sb[:, e, :],
                    start=(e == 0), stop=(e == NE - 1),
                )
            nc.vector.tensor_copy(
                v_sb[:, k, :, 0:64],
                t[:, 0:256].rearrange("p (h c) -> p h c", h=HC))

        def emit_qproj(qc, nch):
            t = psM.tile([128, 512], F32, tag="mi", name=f"qp{qc}_{nch}")
            for e in range(NE):
                nc.tensor.matmul(
                    t[:],
                    wq_sb[:, e, nch * 128:(nch + 1) * 128],
                    xq[qc][e][:],
                    start=(e == 0), stop=(e == NE - 1),
                )
            nc.vector.tensor_copy(qT_sb[nch][:, qc * 512:(qc + 1) * 512], t[:])

        def emit_norm(qc, pair, ovv, qts=(0, 1, 2, 3), split=False):
            # reciprocal of the fused rowsums (col 64 of each 128-col region)
            n = len(qts)
            rt = rpool.tile([128, 2 * n], F32, tag="rt",
                            name=f"rt{qc}_{pair}_{qts[0]}")
            nc.vector.reciprocal(
                rt[:].rearrange("p (q h) -> p q h", q=n),
                ovv[:, qts[0]:qts[0] + n, :, 64])
            nTs = {}
            for i, qt in enumerate(qts):
                nT = npool.tile([128, 128], FP16, tag="nt",
                                name=f"nt{qc}_{pair}_{qt}")
                for h2 in range(2):
                    if split and h2 == 1:
                        # tail only: Act engine is idle, halve the DVE chain
                        nc.scalar.mul(nT[:, 64:128], ovv[:, qt, 1, 0:64],
                                      rt[:, 2 * i + 1:2 * i + 2])
                    else:
                        nc.vector.tensor_scalar(
                            nT[:, h2 * 64:(h2 + 1) * 64],
                            ovv[:, qt, h2, 0:64],
                            rt[:, 2 * i + h2:2 * i + h2 + 1],
                            None, MULT)
                nTs[qt] = nT
            return nTs

        def emit_transpose(qc, pair, qt, nT, pool=None):
            # pool=psS in the tail: scores are done, its slots are idle, and
            # keeping transposes out of the psM ring stops them serializing
            # behind out-proj tiles awaiting their stage copies.
            pool = pool or psM
            tag = {id(psS): "ss", id(psOV): "ov"}.get(id(pool), "mi")
            psT = pool.tile([128, 128], FP16, tag=tag, name=f"tp{qc}_{pair}_{qt}")
            nc.tensor.transpose(psT[:], nT[:], id_sb[:])
            nc.vector.tensor_copy(
                oT2_sb[:, pair, qc * 512 + qt * 128:qc * 512 + (qt + 1) * 128],
                psT[:])

        def emit_outproj(m, split=False, pool=None):
            pool = pool or psM
            tag = {id(psS): "ss", id(psOV): "ov"}.get(id(pool), "mi")
            stage = stpool.tile([128, E], FP16, tag="st", name=f"st{m}")
            for j in range(2):
                t = pool.tile([128, 512], F32, tag=tag, name=f"op{m}_{j}")
                for pair in range(2):
                    nc.tensor.matmul(
                        t[:],
                        oT2_sb[:, pair, m * 128:(m + 1) * 128],
                        wo2_sb[:, pair, j * 512:(j + 1) * 512],
                        start=(pair == 0), stop=(pair == 1),
                    )
                if split and j == 1:
                    nc.scalar.copy(stage[:, 512:1024], t[:])
                else:
                    nc.vector.tensor_copy(stage[:, j * 512:(j + 1) * 512], t[:])
            if split:
                nc.sync.dma_start(out[m * 128:(m + 1) * 128, :], stage[:])
            else:
                nc.gpsimd.dma_start(out[m * 128:(m + 1) * 128, :], stage[:])

        # ---- prefix: K/Q projections, nch-split so pair0 lands fast ----
        # psA (4 banks) coexists with psS (4 banks); pair0's K tiles project
        # and copy out first, Q-qc0 goes through psS slots, and the first
        # scores fire while pair1's K projection finishes in the background.
        pts = {}
        cur = []
        vdone = 0
        with tc.tile_pool(name="psA", bufs=4, space="PSUM") as psA:
            kt0 = [psA.tile([128, 512], F32, tag="mm", name=f"pk0_{m}")
                   for m in range(4)]
            kt1 = [psA.tile([128, 512], F32, tag="mm", name=f"pk1_{m}")
                   for m in range(4)]
            for e in range(NE):
                for m in range(4):
                    nc.tensor.matmul(
                        kt0[m][:], wk_sb[:, e, 0:128],
                        xk[e][:, m * 512:(m + 1) * 512],
                        start=(e == 0), stop=(e == NE - 1))
            for m in range(4):
                nc.vector.tensor_copy(
                    kT_sb[0][:, m * 512:(m + 1) * 512], kt0[m][:])
            # Q-qc0 pair0 chases the xq0 DMAs immediately after K-pair0 (the
            # PE stream is in-order, so nothing may sit between them); all of
            # pair1's projection work weaves into the first scores' shadows.
            tq0 = psS.tile([128, 512], F32, tag="ss", name="pq0_0")
            # tq1 must NOT take a psS ring slot: its reader (the qT1 copy)
            # lands after the early scores, which would deadlock their
            # tile allocations. The psA ring's kt0[0] slot frees early.
            tq1 = psA.tile([128, 512], F32, tag="mm", name="pq0_1")
            for e in range(NE):
                nc.tensor.matmul(
                    tq0[:], wq_sb[:, e, 0:128], xq[0][e][:],
                    start=(e == 0), stop=(e == NE - 1))
            nc.vector.tensor_copy(qT_sb[0][:, 0:512], tq0[:])
            for k in range(6):
                cur.append(emit_scores_exp(0, 0, k))
                if k < 4:     # K-pair1, two e-chunks per score shadow
                    for e in (2 * k, 2 * k + 1):
                        for m in range(4):
                            nc.tensor.matmul(
                                kt1[m][:], wk_sb[:, e, 128:256],
                                xk[e][:, m * 512:(m + 1) * 512],
                                start=(e == 0), stop=(e == NE - 1))
                elif k < 6:   # then Q-pair1, four e-chunks per shadow
                    for e in range(4 * (k - 4), 4 * (k - 3)):
                        nc.tensor.matmul(
                            tq1[:], wq_sb[:, e, 128:256], xq[0][e][:],
                            start=(e == 0), stop=(e == NE - 1))
            for m in range(4):
                nc.vector.tensor_copy(
                    kT_sb[1][:, m * 512:(m + 1) * 512], kt1[m][:])
            for k in range(6, 8):
                cur.append(emit_scores_exp(0, 0, k))
                if k == 6:
                    nc.vector.tensor_copy(qT_sb[1][:, 0:512], tq1[:])
        psOV = ctx.enter_context(tc.tile_pool(name="psOV", bufs=1, space="PSUM"))
        psM = ctx.enter_context(tc.tile_pool(name="psM", bufs=2, space="PSUM"))

        for k in range(8, NK):
            cur.append(emit_scores_exp(0, 0, k))
            if k >= 9 and vdone < 5:   # xv DMAs have landed by these shadows
                emit_vproj(vdone)
                vdone += 1
            elif k == 14:
                emit_qproj(1, 0)
            elif k == 15:
                emit_qproj(1, 1)
        pts[(0, 0)] = cur

        seq = [(qc, pair) for qc in range(NQC) for pair in range(2)]
        trans_pending = None   # (qc, pair, nTs) awaiting transpose weave
        for i, (qc, pair) in enumerate(seq):
            nxt = seq[i + 1] if i + 1 < len(seq) else None
            ps_o = psOV.tile([128, 1024], F32, tag="ov", name=f"ov{qc}_{pair}")
            ovv = ps_o.rearrange("p (q h c) -> p q h c", q=4, h=2)
            cur_pts = pts.pop((qc, pair))
            nxt_pts = [] if nxt else None
            op_ms = list(range((qc - 1) * 4, qc * 4)) if (pair == 0 and qc >= 1) else []
            last_qts = (0, 1, 2, 3)
            for k in range(NK):
                if nxt:
                    nxt_pts.append(emit_scores_exp(nxt[0], nxt[1], k))
                if trans_pending and k in (2, 4, 6, 8):
                    tqc, tpair, tnTs = trans_pending
                    emit_transpose(tqc, tpair, k // 2 - 1, tnTs[k // 2 - 1])
                    if k == 8:
                        trans_pending = None
                if vdone < NK:  # remaining V-proj tiles, just-in-time
                    emit_vproj(vdone)
                    vdone += 1
                emit_pv(pair, k, cur_pts[k], ovv, qts=last_qts)
                if op_ms and k % 4 == 2:
                    emit_outproj(op_ms[k // 4])
                if pair == 0 and 1 <= qc < NQC - 1 and k in (8, 12):
                    # Q projection for the next qc, before its scores appear
                    emit_qproj(qc + 1, (k - 8) // 4)
            if nxt:
                pts[nxt] = nxt_pts
                nTs = emit_norm(qc, pair, ovv)
                trans_pending = (qc, pair, nTs)
            else:
                # tail drain: norm (split across DVE + the idle Act engine),
                # then per-q-tile transpose + out-proj chains distributed
                # over all three PSUM rings so they pipeline
                mb = (NQC - 1) * 4
                nTs = emit_norm(qc, pair, ovv, split=True)
                emit_transpose(qc, pair, 0, nTs[0], pool=psS)
                emit_transpose(qc, pair, 1, nTs[1], pool=psS)
                emit_transpose(qc, pair, 2, nTs[2], pool=psOV)
                emit_transpose(qc, pair, 3, nTs[3], pool=psOV)
                emit_outproj(mb + 0, split=True, pool=psM)
                emit_outproj(mb + 1, split=True, pool=psS)
                emit_outproj(mb + 2, split=True, pool=psM)
                emit_outproj(mb + 3, split=True, pool=psS)

    return nc


_NC_CACHE = {}


def _get_nc():
    if "nc" not in _NC_CACHE:
        _NC_CACHE["nc"] = build()
    return _NC_CACHE["nc"]


def _shard_inputs(query, key, value, Wq, Wk, Wv, Wo):
    """Host-side sharding + layout prep: core c = (batch c//4, head-group c%4)."""
    f16 = np.float16
    xT = []
    for b in range(B):
        xT.append((
            np.ascontiguousarray(query[b].T).astype(f16),
            np.ascontiguousarray(key[b].T).astype(f16),
            np.ascontiguousarray(value[b].T).astype(f16),
        ))
    wT = []
    for g in range(4):
        gc = slice(g * 256, (g + 1) * 256)
        wo_g = Wo[:, gc].T.astype(f16)            # [256, E]
        woT2 = np.ascontiguousarray(
            wo_g.reshape(2, 128, E).transpose(1, 0, 2).reshape(128, 2 * E))
        wT.append((
            np.ascontiguousarray(Wq[gc].T).astype(f16),
            np.ascontiguousarray(Wk[gc].T).astype(f16),
            np.ascontiguousarray(Wv[gc].T).astype(f16),
            woT2,
        ))
    ident = np.eye(128, dtype=f16)
    in_maps = []
    for c in range(NCORES):
        b, g = c // 4, c % 4
        qT, kT, vT = xT[b]
        wq, wk, wv, wo2 = wT[g]
        in_maps.append({
            "xqT": qT, "xkT": kT, "xvT": vT,
            "wqT": wq, "wkT": wk, "wvT": wv, "woT2": wo2,
            "ident": ident,
        })
    return in_maps


def kernel(query, key, value, Wq, Wk, Wv, Wo):
    query = np.asarray(query, dtype=np.float32)
    key = np.asarray(key, dtype=np.float32)
    value = np.asarray(value, dtype=np.float32)
    Wq = np.asarray(Wq, dtype=np.float32)
    Wk = np.asarray(Wk, dtype=np.float32)
    Wv = np.asarray(Wv, dtype=np.float32)
    Wo = np.asarray(Wo, dtype=np.float32)

    nc = _get_nc()
    in_maps = _shard_inputs(query, key, value, Wq, Wk, Wv, Wo)
    res = run_bass_kernel_spmd(nc, in_maps, core_ids=list(range(NCORES)))

    out = np.zeros((B, S, E), dtype=np.float32)
    for c in range(NCORES):
        out[c // 4] += res.results[c]["out"].astype(np.float32)
    return out
